# revision 1
# baseline (speedup 1.0000x reference)
"""Trainium2 Bass kernel for nn_EnhancedGATGCN (GAT -> GCN -> pool -> MLP, + protein conv branch).

Self-contained: host-side sharding prep + 8-core SPMD Bass/Tile device program.

Sharding strategy:
  - Edges (incl. self loops) sorted by dst, partitioned into 8 contiguous dst
    ranges of 2500 nodes; scatter-add is core-local via one-hot matmuls on the
    TensorEngine with PSUM accumulation per 128-dst window.
  - Node feature tables (h for GAT, dinv-scaled h2 for GCN) are computed
    node-sharded, AllGathered as bf16 tables in HBM, and per-edge messages are
    fetched with SWDGE dma_gather (f32 attention logits bit-packed into the
    bf16 rows). GAT softmax denominator rides the scatter matmul as extra rhs
    columns; a_d[dst] is expanded per edge with a transposed-mask matmul; GCN
    sym-norm is factored into per-node dinv scales.
  - Dense tail (conv/MLPs) is data-parallel over the 200-graph batch (25/core)
    and scheduled to overlap the AllGathers / gather-bound stretches.
"""
import os
import sys

import numpy as np

sys.path.insert(0, "/opt/trn_rl_repo")

import ml_dtypes

import concourse.bacc as bacc
import concourse.bass as bass
import concourse.mybir as mybir
import concourse.tile as tile
from concourse.bass_utils import run_bass_kernel_spmd
from concourse.masks import make_identity

F32 = mybir.dt.float32
BF16 = mybir.dt.bfloat16
I16 = mybir.dt.int16
I32 = mybir.dt.int32
AF = mybir.ActivationFunctionType
OP = mybir.AluOpType
BF = ml_dtypes.bfloat16

N, E, B, H, F = 20000, 400000, 200, 10, 78
HID = H * F  # 780
SEQ, VOC, EMB, NF, KS = 1000, 26, 128, 32, 8
CONV_OUT = SEQ - KS + 1  # 993

NCORES = 8
NPC = N // NCORES  # 2500
NPAD = 2560
NWIN = NPAD // 128  # 20
RBF = 896  # bf16 cols per table row; bytes = 1792 (%256==0)
# row: [0:780 h bf16 | 780:800 a_s 10xf32 | 800:896 zeros (820:830 exd scratch on msg tiles)]
GSLOT = 64
MY_G = 32
BPC = B // NCORES  # 25
TOK = BPC * SEQ
TOKPAD = 25600  # 5 groups x 5120 (each: 5 graphs x 1000 tok + 120 pad)
KPAD = 32 * 1024


# ---------------------------------------------------------------- host prep


def _wrap16(idx, epc):
    a = np.zeros((128, epc // 16), np.int16)
    w = idx.reshape(epc // 16, 16).T
    a[:, :] = np.tile(w, (8, 1))
    return a


def host_prep(inputs):
    x = np.asarray(inputs["x"], np.float32)
    edge_index = np.asarray(inputs["edge_index"], np.int64)
    batch = np.asarray(inputs["batch"], np.int64)
    target = np.asarray(inputs["target"], np.int64)

    loops = np.arange(N, dtype=np.int64)
    src = np.concatenate([edge_index[0], loops])
    dst = np.concatenate([edge_index[1], loops])
    order = np.argsort(dst, kind="stable")
    src, dst = src[order], dst[order]

    core_of = dst // NPC
    dst_local = dst - core_of * NPC
    win = dst_local // 128
    maxw = 0
    per_core_edges = []
    for c in range(NCORES):
        m = core_of == c
        s_c, dl_c, w_c = src[m], dst_local[m], win[m]
        per_core_edges.append((s_c, dl_c, w_c))
        maxw = max(maxw, int(np.bincount(w_c, minlength=NWIN).max()))
    tpw = -(-maxw // 128)
    tpw = -(-tpw // 4) * 4
    ntile = NWIN * tpw
    epc = ntile * 128
    nchunk = ntile // 16

    def remap(n):
        return (n // NPC) * NPAD + (n % NPC)

    cores = []
    for c in range(NCORES):
        s_c, dl_c, w_c = per_core_edges[c]
        es = np.zeros(epc, np.int64)
        ew = np.full(epc, -1000.0, np.float32)
        for w in range(NWIN):
            m = w_c == w
            k = int(m.sum())
            o = w * tpw * 128
            es[o : o + k] = s_c[m]
            ew[o : o + k] = (dl_c[m] - w * 128).astype(np.float32)
        cores.append(dict(es=remap(es), ew=ew))

    gat_W = np.asarray(inputs["gat_W"], np.float32)
    wpack0 = np.zeros((78, 1024), np.float32)
    wpack0[:, :HID] = gat_W
    gat_WT_pad = np.zeros((896, 78), np.float32)
    gat_WT_pad[:HID] = gat_W.T
    wasbd = np.zeros((896, 20), np.float32)
    a_src = np.asarray(inputs["gat_a_src"], np.float32)
    a_dst = np.asarray(inputs["gat_a_dst"], np.float32)
    for h in range(H):
        wasbd[h * F : (h + 1) * F, h] = a_src[h]
        wasbd[h * F : (h + 1) * F, 10 + h] = a_dst[h]

    gcn_W_pad = np.zeros((896, HID), np.float32)
    gcn_W_pad[:HID] = np.asarray(inputs["gcn_W"], np.float32)
    fcg1_W_pad = np.zeros((896, 1500), np.float32)
    fcg1_W_pad[:HID] = np.asarray(inputs["fcg1_W"], np.float32)
    fcg2_W_pad = np.zeros((1536, 128), np.float32)
    fcg2_W_pad[:1500] = np.asarray(inputs["fcg2_W"], np.float32)

    fxt_W = np.asarray(inputs["fxt_W"], np.float32)
    fxt_Wp = np.zeros((KPAD, 128), np.float32)
    fxt_Wp.reshape(NF, 1024, 128)[:, :CONV_OUT] = fxt_W.reshape(NF, CONV_OUT, 128)

    cW = np.asarray(inputs["cW"], np.float32)
    cwt = np.ascontiguousarray(cW.transpose(2, 1, 0))  # [8, 128, 32]

    gbase = np.array([batch[c * NPC] for c in range(NCORES)], np.int64)
    span = np.array(
        [batch[min(c * NPC + NPC, N) - 1] - gbase[c] + 1 for c in range(NCORES)]
    )
    assert span.max() <= GSLOT, span.max()
    Cc_all = []
    for c in range(NCORES):
        Cmat = np.zeros((NCORES * GSLOT, MY_G), np.float32)
        for r in range(NCORES):
            for slot in range(GSLOT):
                g = gbase[r] + slot
                col = g - c * BPC
                if 0 <= col < BPC and g < B:
                    Cmat[r * GSLOT + slot, col] = 1.0
        Cc_all.append(Cmat)

    meta = dict(tpw=tpw, ntile=ntile, epc=epc, nchunk=nchunk)

    per_core = []
    for c in range(NCORES):
        ed_ = cores[c]
        bw = np.full(NPAD, -1000.0, np.float32)
        bw[:NPC] = (batch[c * NPC : (c + 1) * NPC] - gbase[c]).astype(np.float32)
        batchw = bw.reshape(NWIN, 128).T.copy()

        tgt = np.zeros(TOKPAD, np.int64)
        tg = target[c * BPC : (c + 1) * BPC].reshape(5, 5 * SEQ)
        tgt.reshape(5, 5120)[:, : 5 * SEQ] = tg

        dstw = ed_["ew"].reshape(ntile, 128).T.copy()

        xTc = np.zeros((78, NPAD), np.float32)
        xTc[:, :NPC] = x[c * NPC : (c + 1) * NPC].T

        d = {
            "xTc": xTc,
            "wpack0": wpack0,
            "gat_WT": gat_WT_pad,
            "wasbd": wasbd,
            "src16": _wrap16(ed_["es"], epc),
            "tgt16": _wrap16(tgt, TOKPAD),
            "dstw": dstw,
            "batchw": batchw,
            "Cc": Cc_all[c],
            "gat_b": np.asarray(inputs["gat_b"], np.float32).reshape(1, HID),
            "gcn_Wp": gcn_W_pad,
            "gcn_b": np.asarray(inputs["gcn_b"], np.float32).reshape(1, HID),
            "fcg1_Wp": fcg1_W_pad,
            "fcg1_b": np.asarray(inputs["fcg1_b"], np.float32).reshape(1, 1500),
            "fcg2_Wp": fcg2_W_pad,
            "fcg2_b": np.asarray(inputs["fcg2_b"], np.float32).reshape(1, 128),
            "emb_bf": np.asarray(inputs["emb"], np.float32).astype(BF),
            "cwt_bf": cwt.astype(BF),
            "cb": np.asarray(inputs["cb"], np.float32).reshape(NF, 1),
            "fxt_Wp": fxt_Wp,
            "fxt_b": np.asarray(inputs["fxt_b"], np.float32).reshape(1, 128),
            "f1_W": np.asarray(inputs["f1_W"], np.float32),
            "f1_b": np.asarray(inputs["f1_b"], np.float32).reshape(1, 1024),
            "f2_W": np.asarray(inputs["f2_W"], np.float32),
            "f2_b": np.asarray(inputs["f2_b"], np.float32).reshape(1, 512),
            "f3_W": np.asarray(inputs["f3_W"], np.float32),
            "f3_b": np.asarray(inputs["f3_b"], np.float32).reshape(1, 256),
            "f4_W": np.asarray(inputs["f4_W"], np.float32),
            "f4_b": np.asarray(inputs["f4_b"], np.float32).reshape(1, 128),
            "o_W": np.asarray(inputs["o_W"], np.float32),
            "o_b": np.asarray(inputs["o_b"], np.float32).reshape(1, 1),
        }
        per_core.append(d)
    return per_core, meta


# ---------------------------------------------------------------- device build

_CACHE = {}


def build_bass(meta):
    PH = int(os.environ.get("KPHASE", "9"))
    key = (meta["tpw"], PH)
    if key in _CACHE:
        return _CACHE[key]

    tpw, ntile, epc, nchunk = meta["tpw"], meta["ntile"], meta["epc"], meta["nchunk"]

    nc = bacc.Bacc(
        "TRN2",
        target_bir_lowering=False,
        debug=False,
        num_devices=NCORES,
        num_swdge_queues=2,
    )

    def inp(name, shape, dt=F32):
        return nc.dram_tensor(name, list(shape), dt, kind="ExternalInput")

    xTc = inp("xTc", (78, NPAD))
    wpack0 = inp("wpack0", (78, 1024))
    gat_WT = inp("gat_WT", (896, 78))
    wasbd = inp("wasbd", (896, 20))
    src16 = inp("src16", (128, epc // 16), I16)
    tgt16 = inp("tgt16", (128, TOKPAD // 16), I16)
    dstw = inp("dstw", (128, ntile))
    batchw = inp("batchw", (128, NWIN))
    Cc = inp("Cc", (NCORES * GSLOT, MY_G))
    gat_b = inp("gat_b", (1, HID))
    gcn_Wp = inp("gcn_Wp", (896, HID))
    gcn_b = inp("gcn_b", (1, HID))
    fcg1_Wp = inp("fcg1_Wp", (896, 1500))
    fcg1_b = inp("fcg1_b", (1, 1500))
    fcg2_Wp = inp("fcg2_Wp", (1536, 128))
    fcg2_b = inp("fcg2_b", (1, 128))
    emb_bf = inp("emb_bf", (VOC, EMB), BF16)
    cwt_bf = inp("cwt_bf", (KS, EMB, NF), BF16)
    cb = inp("cb", (NF, 1))
    fxt_Wp = inp("fxt_Wp", (KPAD, 128))
    fxt_b = inp("fxt_b", (1, 128))
    f1_W = inp("f1_W", (256, 1024))
    f1_b = inp("f1_b", (1, 1024))
    f2_W = inp("f2_W", (1024, 512))
    f2_b = inp("f2_b", (1, 512))
    f3_W = inp("f3_W", (512, 256))
    f3_b = inp("f3_b", (1, 256))
    f4_W = inp("f4_W", (256, 128))
    f4_b = inp("f4_b", (1, 128))
    o_W = inp("o_W", (128, 1))
    o_b = inp("o_b", (1, 1))
    out_d = nc.dram_tensor("out", [MY_G, 1], F32, kind="ExternalOutput")
    KDEBUG = bool(int(os.environ.get("KDEBUG", "0")))
    if KDEBUG:
        out_x1 = nc.dram_tensor("out_x1", [NPAD, HID], F32, kind="ExternalOutput")
        out_h = nc.dram_tensor("out_h", [256, RBF], BF16, kind="ExternalOutput")
        out_adw = nc.dram_tensor("out_adw", [128, NWIN * 10], BF16, kind="ExternalOutput")
        out_xt = nc.dram_tensor("out_xt", [MY_G, 128], F32, kind="ExternalOutput")

    hin = nc.dram_tensor("hin", [NPAD, RBF], BF16)
    htabG = nc.dram_tensor("htabG", [NCORES * NPAD, RBF], BF16, addr_space="Shared")
    x1d = nc.dram_tensor("x1d", [NPAD, HID], F32)
    agin = nc.dram_tensor("agin", [NPAD, RBF], BF16)
    htab2G = nc.dram_tensor("htab2G", [NCORES * NPAD, RBF], BF16, addr_space="Shared")
    poolin = nc.dram_tensor("poolin", [GSLOT, HID], F32)
    poolall = nc.dram_tensor("poolall", [NCORES * GSLOT, HID], F32, addr_space="Shared")

    RG = [list(range(NCORES))]

    with tile.TileContext(nc) as tc:
        import contextlib

        ctx = contextlib.ExitStack()
        with ctx:
            pers = ctx.enter_context(tc.tile_pool(name="pers", bufs=1))

            # consts
            iota_i = pers.tile([128, 128], I32)
            nc.gpsimd.iota(iota_i[:], pattern=[[1, 128]], base=0, channel_multiplier=0)
            iota_f = pers.tile([128, 128], F32)
            nc.vector.tensor_copy(iota_f[:], iota_i[:])
            ident = pers.tile([128, 128], F32)
            make_identity(nc, ident[:])
            ident_bf = pers.tile([128, 128], BF16)
            nc.vector.tensor_copy(ident_bf[:], ident[:])
            ones1 = pers.tile([1, 128], F32)
            nc.gpsimd.memset(ones1[:], 1.0)

            bias_tiles = {}
            with tc.tile_pool(name="psB", bufs=1, space="PSUM") as psB:

                def bcast_bias(dram, width, name):
                    t = pers.tile([128, width], F32, tag=f"bc_{name}")
                    row = pers.tile([1, width], F32, tag=f"br_{name}")
                    nc.sync.dma_start(row[:], dram[0:1, :])
                    for n0 in range(0, width, 512):
                        nn = min(512, width - n0)
                        ps = psB.tile([128, 512], F32, space="PSUM", tag="bcps")
                        nc.tensor.matmul(
                            ps[:, :nn], lhsT=ones1[:], rhs=row[:, n0 : n0 + nn],
                            start=True, stop=True,
                        )
                        nc.any.tensor_copy(t[:, n0 : n0 + nn], ps[:, :nn])
                    return t

                gatb_bc = bcast_bias(gat_b, HID, "gatb")
                gcnb_bc = bcast_bias(gcn_b, HID, "gcnb")
                fcg1b_bc = bcast_bias(fcg1_b, 1500, "fcg1b")
                fcg2b_bc = bcast_bias(fcg2_b, 128, "fcg2b")
                fxtb_bc = bcast_bias(fxt_b, 128, "fxtb")
                f1b_bc = bcast_bias(f1_b, 1024, "f1b")
                f2b_bc = bcast_bias(f2_b, 512, "f2b")
                f3b_bc = bcast_bias(f3_b, 256, "f3b")
                f4b_bc = bcast_bias(f4_b, 128, "f4b")
                ob_bc = bcast_bias(o_b, 1, "ob")

            # residents
            dstw_t = pers.tile([128, ntile], F32)
            nc.sync.dma_start(dstw_t[:], dstw[:, :])
            batchw_t = pers.tile([128, NWIN], F32)
            nc.sync.dma_start(batchw_t[:], batchw[:, :])
            src_t = pers.tile([128, epc // 16], I16)
            nc.sync.dma_start(src_t[:], src16[:, :])
            tgt_t = pers.tile([128, TOKPAD // 16], I16)
            nc.sync.dma_start(tgt_t[:], tgt16[:, :])

            dinv_all = pers.tile([128, NWIN], F32)
            deg_all = pers.tile([128, NWIN], F32)
            adw_all = pers.tile([128, NWIN, 10], BF16)
            xt_sb = pers.tile([MY_G, 128], F32)
            nc.gpsimd.memset(xt_sb[:], 0.0)

            # shared edge-phase machinery -----------------------------------
            def edge_phase(table, gat, x_epilogue, agg_bufs):
                with (
                    tc.tile_pool(name="msgp", bufs=3) as msgp,
                    tc.tile_pool(name="smallp", bufs=2) as smallp,
                    tc.tile_pool(name="maskp", bufs=6) as maskp,
                    tc.tile_pool(name="epip", bufs=2) as epip,
                    tc.tile_pool(name="psA", bufs=agg_bufs, space="PSUM") as psA,
                    tc.tile_pool(name="psS", bufs=2, space="PSUM") as psS,
                ):
                    aggp = None
                    for c in range(nchunk):
                        isl = slice(c * 128, (c + 1) * 128)
                        msg = msgp.tile([128, 16, RBF], BF16, tag="msg")
                        nc.gpsimd.dma_gather(
                            msg[:],
                            table.ap()[:, 0:RBF],
                            src_t[:, isl],
                            num_idxs=2048,
                            num_idxs_reg=2048,
                            elem_size=RBF,
                            elem_step=RBF,
                            single_packet=False,
                        )
                        maskall = maskp.tile([128, 16, 128], BF16, tag="maskall")
                        if gat:
                            nc.gpsimd.memset(msg[:, :, 830:831], 1.0)
                            sall = smallp.tile([128, 16, 10], F32, tag="sall")
                        # pass 1 (per 4-tile group): masks, a_d expansion,
                        # leaky+exp, message scaling
                        for q4 in range(4):
                            jsl = slice(q4 * 4, q4 * 4 + 4)
                            for i in range(4):
                                g = c * 16 + q4 * 4 + i
                                nc.vector.tensor_tensor(
                                    maskall[:, q4 * 4 + i, :],
                                    dstw_t[:, g : g + 1].to_broadcast([128, 128]),
                                    iota_f[:],
                                    op=OP.is_equal,
                                )
                            if not gat:
                                continue
                            trT = psS.tile([128, 512], BF16, space="PSUM", tag="trT")
                            for i in range(4):
                                nc.tensor.transpose(
                                    trT[:, i * 128 : (i + 1) * 128],
                                    maskall[:, q4 * 4 + i, :],
                                    ident_bf[:],
                                )
                            maskT = maskp.tile([128, 4, 128], BF16, tag="maskT")
                            nc.scalar.copy(maskT[:], trT[:])
                            adx = psS.tile([128, 512], F32, space="PSUM", tag="adx")
                            for i in range(4):
                                nc.tensor.matmul(
                                    adx[:, i * 10 : i * 10 + 10],
                                    lhsT=maskT[:, i, :],
                                    rhs=adw_all[:, (c * 16 + q4 * 4 + i) // tpw, :],
                                    start=True,
                                    stop=True,
                                )
                            nc.vector.tensor_tensor(
                                sall[:, jsl, :],
                                msg[:, jsl, 780:800].bitcast(F32),
                                adx[:, 0:40].rearrange("p (a b) -> p a b", b=10),
                                op=OP.add,
                            )
                            s2 = smallp.tile([128, 4, 10], F32, tag="s2")
                            nc.vector.tensor_scalar_mul(s2[:], sall[:, jsl, :], 0.2)
                            nc.vector.tensor_tensor(
                                sall[:, jsl, :], sall[:, jsl, :], s2[:], op=OP.max
                            )
                            nc.scalar.activation(
                                msg[:, jsl, 820:830], sall[:, jsl, :], AF.Exp
                            )
                            nc.vector.tensor_tensor(
                                msg[:, jsl, 0:HID].rearrange(
                                    "p c (h f) -> p c h f", h=H
                                ),
                                msg[:, jsl, 0:HID].rearrange(
                                    "p c (h f) -> p c h f", h=H
                                ),
                                msg[:, jsl, 820:830, None].to_broadcast([128, 4, H, F]),
                                op=OP.mult,
                            )
                        # pass 2: scatter matmuls
                        for j in range(16):
                            g = c * 16 + j
                            w = g // tpw
                            first = g % tpw == 0
                            last = g % tpw == tpw - 1
                            if first:
                                aggp = psA.tile(
                                    [128, 1024], F32, space="PSUM", tag="aggp"
                                )
                            n_hi = 831 if gat else HID
                            for n0, nn in ((0, 512), (512, n_hi - 512)):
                                nc.tensor.matmul(
                                    aggp[:, n0 : n0 + nn],
                                    lhsT=maskall[:, j, :],
                                    rhs=msg[:, j, n0 : n0 + nn],
                                    start=first,
                                    stop=last,
                                )
                            if last:
                                x_epilogue(w, aggp, epip)

                    return

            def gat_epilogue(w, aggp, epip):
                rec = epip.tile([128, 12], F32, tag="rec")
                nc.vector.tensor_scalar_add(rec[:, 0:11], aggp[:, 820:831], 1e-20)
                nc.vector.tensor_copy(deg_all[:, w : w + 1], rec[:, 10:11])
                rcp = epip.tile([128, 10], F32, tag="rcp")
                nc.vector.reciprocal(rcp[:], rec[:, 0:10])
                x1w = epip.tile([128, HID], F32, tag="x1w")
                nc.vector.tensor_tensor(
                    x1w[:].rearrange("p (h f) -> p h f", h=H),
                    aggp[:, 0:HID].rearrange("p (h f) -> p h f", h=H),
                    rcp[:, :, None].to_broadcast([128, H, F]),
                    op=OP.mult,
                )
                nc.vector.tensor_tensor(x1w[:], x1w[:], gatb_bc[:], op=OP.add)
                nc.vector.tensor_scalar_max(x1w[:], x1w[:], 0.0)
                nc.sync.dma_start(x1d.ap()[w * 128 : (w + 1) * 128, :], x1w[:])

            # ---- phase 1: own h rows; AllGather table ----
            if PH >= 1:
              with (
                tc.tile_pool(name="p1", bufs=1) as p1,
                tc.tile_pool(name="p1h", bufs=3) as p1h,
                tc.tile_pool(name="ps1", bufs=1, space="PSUM") as ps1,
              ):
                xT_sb = p1.tile([78, NPAD], F32)
                nc.sync.dma_start(xT_sb[:], xTc[:, :])
                wp_sb = p1.tile([78, 1024], F32)
                nc.sync.dma_start(wp_sb[:], wpack0[:, :])
                gwt_sb = p1.tile([128, 7, 78], F32)
                nc.sync.dma_start(
                    gwt_sb[:], gat_WT.ap().rearrange("(c p) f -> p c f", p=128)
                )
                was_sb = p1.tile([128, 7, 20], F32)
                nc.sync.dma_start(
                    was_sb[:], wasbd.ap().rearrange("(c p) f -> p c f", p=128)
                )
                wcps = ps1.tile([78, 512], F32, space="PSUM", tag="wcps")
                for kc in range(7):
                    nc.tensor.matmul(
                        wcps[:, 0:20],
                        lhsT=gwt_sb[:, kc, :],
                        rhs=was_sb[:, kc, :],
                        start=(kc == 0),
                        stop=(kc == 6),
                    )
                nc.any.tensor_copy(wp_sb[:, HID : HID + 20], wcps[:, 0:20])

                for t in range(NWIN):
                    hp = ps1.tile([128, 1024], F32, space="PSUM", tag="hp")
                    for n0 in (0, 512):
                        nc.tensor.matmul(
                            hp[:, n0 : n0 + 512],
                            lhsT=xT_sb[:, t * 128 : (t + 1) * 128],
                            rhs=wp_sb[:, n0 : n0 + 512],
                            start=True,
                            stop=True,
                        )
                    hrow = p1h.tile([128, RBF], BF16, tag="hrow")
                    nc.vector.tensor_copy(hrow[:, 0:HID], hp[:, 0:HID])
                    nc.vector.tensor_copy(
                        hrow[:, 780:800].bitcast(F32), hp[:, 780:790]
                    )
                    nc.gpsimd.memset(hrow[:, 800:RBF], 0.0)
                    nc.vector.tensor_copy(adw_all[:, t, :], hp[:, 790:800])
                    nc.sync.dma_start(
                        hin.ap()[t * 128 : (t + 1) * 128, :], hrow[:]
                    )
                nc.gpsimd.collective_compute(
                    "AllGather",
                    OP.bypass,
                    replica_groups=RG,
                    ins=[hin.ap().opt()],
                    outs=[htabG.ap().opt()],
                )

            # ---- phase 2: GAT ----
            if PH >= 2:
                edge_phase(htabG, True, gat_epilogue, agg_bufs=2)
                nc.scalar.activation(dinv_all[:], deg_all[:], AF.Sqrt)
                nc.vector.tensor_scalar_add(dinv_all[:], dinv_all[:], 1e-20)
                nc.vector.reciprocal(dinv_all[:], dinv_all[:])

            # ================== protein scope (overlaps phase 3 + AllGathers) =========
            with (
                tc.tile_pool(name="pp", bufs=1) as pp,
                tc.tile_pool(name="ppg", bufs=2) as ppg,
                tc.tile_pool(name="ppw", bufs=2) as ppw,
                tc.tile_pool(name="psC", bufs=2, space="PSUM") as psC,
                tc.tile_pool(name="psTr", bufs=1, space="PSUM") as psTr,
                tc.tile_pool(name="psX", bufs=1, space="PSUM") as psX,
            ):
                # ---- phase 3: h2s + AllGather ----
                if PH >= 3:
                  with (
                    tc.tile_pool(name="p3", bufs=2) as p3,
                    tc.tile_pool(name="p3w", bufs=1) as p3w,
                    tc.tile_pool(name="psT3", bufs=1, space="PSUM") as psT3,
                    tc.tile_pool(name="psH3", bufs=1, space="PSUM") as psH3,
                  ):
                    gcnw_sb = p3w.tile([128, 7, HID], F32)
                    nc.sync.dma_start(
                        gcnw_sb[:], gcn_Wp.ap().rearrange("(c p) f -> p c f", p=128)
                    )
                    for t in range(NWIN):
                        x1t = p3.tile([128, HID], F32, tag="x1t")
                        nc.sync.dma_start(x1t[:], x1d.ap()[t * 128 : (t + 1) * 128, :])
                        x1T = p3.tile([128, 7, 128], F32, tag="x1T")
                        for kc in range(7):
                            sz = 128 if kc < 6 else 12
                            trp = psT3.tile([128, 128], F32, space="PSUM", tag="trp")
                            nc.tensor.transpose(
                                trp[0:sz, :], x1t[:, kc * 128 : kc * 128 + sz], ident[:]
                            )
                            nc.any.tensor_copy(x1T[0:sz, kc, :], trp[0:sz, :])
                        h2ps = psH3.tile([128, 1024], F32, space="PSUM", tag="h2ps")
                        for n0, nn in ((0, 512), (512, 268)):
                            for kc in range(7):
                                sz = 128 if kc < 6 else 12
                                nc.tensor.matmul(
                                    h2ps[:, n0 : n0 + nn],
                                    lhsT=x1T[0:sz, kc, :],
                                    rhs=gcnw_sb[0:sz, kc, n0 : n0 + nn],
                                    start=(kc == 0),
                                    stop=(kc == 6),
                                )
                        h2s = p3.tile([128, RBF], BF16, tag="h2s")
                        nc.vector.tensor_tensor(
                            h2s[:, 0:HID],
                            h2ps[:, 0:HID],
                            dinv_all[:, t : t + 1].to_broadcast([128, HID]),
                            op=OP.mult,
                        )
                        nc.gpsimd.memset(h2s[:, HID:RBF], 0.0)
                        nc.sync.dma_start(agin.ap()[t * 128 : (t + 1) * 128, :], h2s[:])
                    nc.gpsimd.collective_compute(
                        "AllGather",
                        OP.bypass,
                        replica_groups=RG,
                        ins=[agin.ap().opt()],
                        outs=[htab2G.ap().opt()],
                    )

                # ---- protein branch (gap-filler; no deps on graph phases) ----
                if PH >= 4:
                    cwt_sb = pp.tile([128, KS, NF], BF16)
                    nc.sync.dma_start(
                        cwt_sb[:], cwt_bf.ap().rearrange("k p o -> p k o")
                    )
                    cb_sb = pp.tile([NF, 1], F32)
                    nc.sync.dma_start(cb_sb[:], cb.ap()[:, :])
                    cT_all = pp.tile([128, 8, NF, BPC], F32)

                    for grp in range(5):
                        gt = ppg.tile([128, 40, 128], BF16, tag="embg")
                        nc.gpsimd.dma_gather(
                            gt[:],
                            emb_bf.ap()[:, :],
                            tgt_t[:, grp * 320 : (grp + 1) * 320],
                            num_idxs=5120,
                            num_idxs_reg=5120,
                            elem_size=128,
                            elem_step=128,
                            single_packet=False,
                        )
                        et5 = ppg.tile([128, 5120], BF16, tag="et5")
                        for i in range(40):
                            trp = psTr.tile([128, 128], BF16, space="PSUM", tag="trp2")
                            nc.tensor.transpose(trp[:], gt[:, i, :], ident_bf[:])
                            nc.any.tensor_copy(et5[:, i * 128 : (i + 1) * 128], trp[:])
                        for bl in range(5):
                            b = grp * 5 + bl
                            boff = bl * 1000
                            csb = pp.tile([NF, 1024], F32, tag="csb")
                            for p0 in (0, 512):
                                cps = psC.tile([NF, 512], F32, space="PSUM", tag="cps")
                                for k in range(KS):
                                    nc.tensor.matmul(
                                        cps[:, 0:512],
                                        lhsT=cwt_sb[:, k, :],
                                        rhs=et5[:, boff + k + p0 : boff + k + p0 + 512],
                                        start=(k == 0),
                                        stop=(k == KS - 1),
                                    )
                                nc.scalar.activation(
                                    csb[:, p0 : p0 + 512], cps[:, 0:512],
                                    AF.Identity, bias=cb_sb[:, 0:1],
                                )
                            for pc in range(8):
                                trc = psTr.tile(
                                    [128, 128], F32, space="PSUM", tag="trc"
                                )
                                nc.tensor.transpose(
                                    trc[:, 0:NF],
                                    csb[:, pc * 128 : (pc + 1) * 128],
                                    ident[0:NF, 0:NF],
                                )
                                nc.any.tensor_copy(cT_all[:, pc, :, b], trc[:, 0:NF])

                    xtps = psX.tile([MY_G, 128], F32, space="PSUM", tag="xtps")
                    for sc in range(16):
                        wpt = ppw.tile([128, 16, 128], F32, tag="wpt")
                        nc.sync.dma_start(
                            wpt[:],
                            fxt_Wp.ap()[sc * 2048 : (sc + 1) * 2048, :].rearrange(
                                "(c p) j -> p c j", p=128
                            ),
                        )
                        for sub in range(16):
                            q = sc * 16 + sub
                            o, t8 = q // 8, q % 8
                            nc.tensor.matmul(
                                xtps[0:BPC, :],
                                lhsT=cT_all[:, t8, o, :],
                                rhs=wpt[:, sub, :],
                                start=(q == 0),
                                stop=(q == 255),
                            )
                    nc.vector.tensor_tensor(
                        xt_sb[0:BPC, :], xtps[0:BPC, :], fxtb_bc[0:BPC, :], op=OP.add
                    )

            # ================== phase 4: GCN + pooling; phase 5: head ========
            if PH >= 5:
              with tc.tile_pool(name="psP", bufs=1, space="PSUM") as psP:
                poolps = psP.tile([GSLOT, 1024], F32, space="PSUM", tag="poolps")

                def gcn_epilogue(w, aggp, epip):
                    x2w = epip.tile([128, HID], F32, tag="x2w")
                    nc.vector.tensor_tensor(
                        x2w[:],
                        aggp[:, 0:HID],
                        dinv_all[:, w : w + 1].to_broadcast([128, HID]),
                        op=OP.mult,
                    )
                    nc.vector.tensor_tensor(x2w[:], x2w[:], gcnb_bc[:], op=OP.add)
                    nc.vector.tensor_scalar_max(x2w[:], x2w[:], 0.0)
                    ph = epip.tile([128, GSLOT], F32, tag="poolhot")
                    nc.vector.tensor_tensor(
                        ph[:],
                        batchw_t[:, w : w + 1].to_broadcast([128, GSLOT]),
                        iota_f[:, 0:GSLOT],
                        op=OP.is_equal,
                    )
                    for n0, nn in ((0, 512), (512, 268)):
                        nc.tensor.matmul(
                            poolps[:, n0 : n0 + nn],
                            lhsT=ph[:],
                            rhs=x2w[:, n0 : n0 + nn],
                            start=(w == 0),
                            stop=(w == NWIN - 1),
                        )

                edge_phase(htab2G, False, gcn_epilogue, agg_bufs=2)
                poolsb = pers.tile([GSLOT, HID], F32)
                nc.any.tensor_copy(poolsb[:], poolps[:, 0:HID])

              with (
                    tc.tile_pool(name="p5", bufs=1) as p5,
                    tc.tile_pool(name="p5w", bufs=2) as p5w,
                    tc.tile_pool(name="ps5", bufs=2, space="PSUM") as ps5,
                    tc.tile_pool(name="ps5t", bufs=2, space="PSUM") as ps5t,
                ):
                    nc.sync.dma_start(poolin.ap()[:, :], poolsb[:])
                    nc.gpsimd.collective_compute(
                        "AllGather",
                        OP.bypass,
                        replica_groups=RG,
                        ins=[poolin.ap().opt()],
                        outs=[poolall.ap().opt()],
                    )
                    Cc_sb = p5.tile([128, 4, MY_G], F32)
                    nc.sync.dma_start(
                        Cc_sb[:], Cc.ap().rearrange("(c p) g -> p c g", p=128)
                    )
                    pall = p5.tile([128, 4, HID], F32)
                    nc.sync.dma_start(
                        pall[:], poolall.ap().rearrange("(c p) f -> p c f", p=128)
                    )
                    xgps = ps5.tile([MY_G, 1024], F32, space="PSUM", tag="mlp_ps")
                    for kc in range(4):
                        for n0, nn in ((0, 512), (512, 268)):
                            nc.tensor.matmul(
                                xgps[:, n0 : n0 + nn],
                                lhsT=Cc_sb[:, kc, :],
                                rhs=pall[:, kc, n0 : n0 + nn],
                                start=(kc == 0),
                                stop=(kc == 3),
                            )
                    xg = p5.tile([MY_G, HID], F32, tag="act0")
                    nc.any.tensor_copy(xg[:], xgps[:, 0:HID])

                    def dense(x_sb, k_real, w_dram, w_rows, n_out, b_bc, relu, tag):
                        nkc = (k_real + 127) // 128
                        xT_t = p5.tile([128, nkc, MY_G], F32, tag="xT5")
                        for kc in range(nkc):
                            sz = min(128, k_real - kc * 128)
                            trp = ps5t.tile([128, MY_G], F32, space="PSUM", tag="tr5")
                            nc.tensor.transpose(
                                trp[0:sz, :],
                                x_sb[:, kc * 128 : kc * 128 + sz],
                                ident[0:MY_G, 0:MY_G],
                            )
                            nc.any.tensor_copy(xT_t[0:sz, kc, :], trp[0:sz, :])
                        w_sb = p5w.tile([128, w_rows // 128, n_out], F32, tag="w5")
                        nc.sync.dma_start(
                            w_sb[:], w_dram.ap().rearrange("(c p) f -> p c f", p=128)
                        )
                        yps = ps5.tile([MY_G, 1536], F32, space="PSUM", tag="mlp_ps")
                        for n0 in range(0, n_out, 512):
                            nn = min(512, n_out - n0)
                            for kc in range(nkc):
                                sz = min(128, k_real - kc * 128)
                                nc.tensor.matmul(
                                    yps[:, n0 : n0 + nn],
                                    lhsT=xT_t[0:sz, kc, :],
                                    rhs=w_sb[0:sz, kc, n0 : n0 + nn],
                                    start=(kc == 0),
                                    stop=(kc == nkc - 1),
                                )
                        y = p5.tile([MY_G, n_out], F32, tag="y5")
                        nc.vector.tensor_tensor(
                            y[:], yps[:, 0:n_out], b_bc[0:MY_G, 0:n_out], op=OP.add
                        )
                        if relu:
                            nc.vector.tensor_scalar_max(y[:], y[:], 0.0)
                        return y

                    y1 = dense(xg, HID, fcg1_Wp, 896, 1500, fcg1b_bc, True, "fcg1")
                    xgo = dense(y1, 1500, fcg2_Wp, 1536, 128, fcg2b_bc, False, "fcg2")
                    xc = p5.tile([MY_G, 256], F32, tag="xc")
                    nc.any.tensor_copy(xc[:, 0:128], xgo[:])
                    nc.any.tensor_copy(xc[:, 128:256], xt_sb[:])
                    a1 = dense(xc, 256, f1_W, 256, 1024, f1b_bc, True, "f1")
                    a2 = dense(a1, 1024, f2_W, 1024, 512, f2b_bc, True, "f2")
                    a3 = dense(a2, 512, f3_W, 512, 256, f3b_bc, True, "f3")
                    a4 = dense(a3, 256, f4_W, 256, 128, f4b_bc, True, "f4")
                    yo = dense(a4, 128, o_W, 128, 1, ob_bc, False, "o")
                    nc.sync.dma_start(out_d.ap()[:, :], yo[:])
                    if KDEBUG:
                        dbg = p5.tile([128, 2, RBF], BF16, tag="dbg")
                        nc.sync.dma_start(
                            dbg[:], htabG.ap()[0:256, :].rearrange("(c p) f -> p c f", p=128)
                        )
                        nc.sync.dma_start(
                            out_h.ap().rearrange("(c p) f -> p c f", p=128), dbg[:]
                        )
                        dbg2 = p5.tile([128, NWIN, HID], F32, tag="dbg2")
                        nc.sync.dma_start(
                            dbg2[:], x1d.ap().rearrange("(c p) f -> p c f", p=128)
                        )
                        nc.sync.dma_start(
                            out_x1.ap().rearrange("(c p) f -> p c f", p=128), dbg2[:]
                        )
                        nc.sync.dma_start(
                            out_adw.ap(), adw_all[:].rearrange("p a b -> p (a b)")
                        )
                        nc.sync.dma_start(out_xt.ap()[:, :], xt_sb[:])

    nc.compile()
    _CACHE[key] = nc
    return nc


# ---------------------------------------------------------------- entry point


def _ensure_ntff_hook():
    """Install antenv.axon_hooks + register the ctypes NTFF hook if the image
    lacks them (profiling only; failures are non-fatal)."""
    import types

    try:
        import antenv.axon_hooks  # noqa: F401

        if antenv.axon_hooks.get_axon_ntff_profile_hook() is not None:
            return
    except ImportError:
        import antenv

        mod = types.ModuleType("antenv.axon_hooks")
        mod._hook = None

        def set_axon_ntff_profile_hook(h, _m=mod):
            _m._hook = h

        def get_axon_ntff_profile_hook(_m=mod):
            return _m._hook

        mod.set_axon_ntff_profile_hook = set_axon_ntff_profile_hook
        mod.get_axon_ntff_profile_hook = get_axon_ntff_profile_hook
        sys.modules["antenv.axon_hooks"] = mod
        antenv.axon_hooks = mod
    try:
        from antenv.axon_hooks import set_axon_ntff_profile_hook as _set
        from trn_agent_boot.trn_boot import _ntff_profile_via_ctypes

        hook = _ntff_profile_via_ctypes("/opt/axon/libaxon_pjrt.so")
        if hook is not None:
            _set(hook)
    except Exception:
        pass


def _enable_ldw_opt():
    """Turn on walrus's LDWEIGHTS dedup pass (consecutive matmuls sharing a
    stationary operand skip the reload). Opt-in via KLDWOPT=1."""
    import concourse.bass_utils as bu

    if getattr(bu, "_ldw_patched", False):
        return
    orig = bu.run_command

    def patched(argv, **kw):
        argv = [
            "--enable-ldw-opt=true" if a == "--enable-ldw-opt=false" else a
            for a in argv
        ]
        return orig(argv, **kw)

    bu.run_command = patched
    bu._ldw_patched = True


def kernel(**inputs) -> np.ndarray:
    if bool(int(os.environ.get("KLDWOPT", "0"))):
        _enable_ldw_opt()
    per_core, meta = host_prep(inputs)
    nc = build_bass(meta)
    in_maps = [{k: np.ascontiguousarray(v) for k, v in d.items()} for d in per_core]
    trace = bool(int(os.environ.get("KERNEL_TRACE", "0")))
    if trace:
        _ensure_ntff_hook()
    res = run_bass_kernel_spmd(nc, in_maps, core_ids=list(range(NCORES)), trace=trace)
    if trace and res.exec_time_ns is not None:
        print(f"HW exec time: {res.exec_time_ns} ns")
        kernel.last_exec_ns = res.exec_time_ns
    out = np.concatenate([res.results[c]["out"][:BPC] for c in range(NCORES)], 0)
    return out.astype(np.float32)



# revision 20
# speedup vs baseline: 1.5131x; 1.5131x over previous
"""Trainium2 Bass kernel for nn_EnhancedGATGCN (GAT -> GCN -> pool -> MLP, + protein conv branch).

Self-contained: host-side sharding prep + 8-core SPMD Bass/Tile device program.

Sharding strategy (v2):
  - Edges (self loops handled locally) sorted by dst, partitioned into 8
    contiguous dst ranges of 2500 nodes; scatter-add is core-local via one-hot
    matmuls on the TensorEngine with PSUM accumulation per 128-dst window.
  - Node feature tables (h for GAT, dinv-scaled h2 for GCN) are computed
    node-sharded in (f,h)-major feature layout, AllGathered as bf16 tables in
    HBM in two halves (windows 0-9 / 10-19) so the first half overlaps
    compute; per-edge messages are fetched with SWDGE dma_gather.
  - GCN input h2 = relu(x1) @ gcn_W is computed inside the GAT epilogue per
    window (no DRAM bounce), so AllGather #2 starts as soon as the GAT edge
    phase drains.
  - The protein branch (embedding via host-built one-hot matmul + conv + fxt)
    runs at the front, hidden under phase 1 + AllGather #1.
  - Dense tail is data-parallel over the 200-graph batch (25/core).
"""
import os
import sys

import numpy as np

sys.path.insert(0, "/opt/trn_rl_repo")

import ml_dtypes

import concourse.bacc as bacc
import concourse.bass as bass
import concourse.mybir as mybir
import concourse.tile as tile
from concourse.bass_utils import run_bass_kernel_spmd
from concourse.masks import make_identity

F32 = mybir.dt.float32
BF16 = mybir.dt.bfloat16
I16 = mybir.dt.int16
I32 = mybir.dt.int32
AF = mybir.ActivationFunctionType
OP = mybir.AluOpType
BF = ml_dtypes.bfloat16

N, E, B, H, F = 20000, 400000, 200, 10, 78
HID = H * F  # 780
SEQ, VOC, EMB, NF, KS = 1000, 26, 128, 32, 8
CONV_OUT = SEQ - KS + 1  # 993

NCORES = 8
NPC = N // NCORES  # 2500
NPAD = 2560
NWIN = NPAD // 128  # 20
HALFW = 10  # windows per table half
HALFR = HALFW * 128  # 1280 rows per core per half
SEG = NCORES * HALFR  # 10240 rows per table segment
RBF = 896  # bf16 cols per table row; bytes = 1792 (%256==0)
# GAT row: [0:780 h bf16 (f,h)-major | 780:800 a_s 10xf32 | 800 one | 801:896 zeros]
# during edge pass 780:790 is overwritten with exp(leaky(e)) per edge
GSLOT = 64
MY_G = 32
BPC = B // NCORES  # 25
TOKPAD = 25600  # 5 groups x 5120 (each: 5 graphs x 1000 tok + 120 pad)
KPAD = 32 * 1024


# ---------------------------------------------------------------- host prep


def _wrap16(idx, epc):
    a = np.zeros((128, epc // 16), np.int16)
    w = idx.reshape(epc // 16, 16).T
    a[:, :] = np.tile(w, (8, 1))
    return a


def host_prep(inputs):
    x = np.asarray(inputs["x"], np.float32)
    edge_index = np.asarray(inputs["edge_index"], np.int64)
    batch = np.asarray(inputs["batch"], np.int64)
    target = np.asarray(inputs["target"], np.int64)

    src = edge_index[0]
    dst = edge_index[1]
    order = np.argsort(dst, kind="stable")
    src, dst = src[order], dst[order]

    core_of = dst // NPC
    dst_local = dst - core_of * NPC
    win = dst_local // 128
    maxw = 0
    per_core_edges = []
    for c in range(NCORES):
        m = core_of == c
        s_c, dl_c, w_c = src[m], dst_local[m], win[m]
        per_core_edges.append((s_c, dl_c, w_c))
        maxw = max(maxw, int(np.bincount(w_c, minlength=NWIN).max()))
    tpw = -(-maxw // 128)
    ntile = NWIN * tpw
    epc = ntile * 128

    def remap(n):
        c = n // NPC
        l = n % NPC
        return np.where(l < HALFR, c * HALFR + l, SEG + c * HALFR + (l - HALFR))

    cores = []
    for c in range(NCORES):
        s_c, dl_c, w_c = per_core_edges[c]
        es = np.zeros(epc, np.int64)
        ew = np.full(epc, -1000.0, np.float32)
        for w in range(NWIN):
            m = w_c == w
            k = int(m.sum())
            o = w * tpw * 128
            es[o : o + k] = s_c[m]
            ew[o : o + k] = (dl_c[m] - w * 128).astype(np.float32)
        mT = np.zeros((128, epc), np.float32)
        valid = ew >= 0
        mT[ew[valid].astype(np.int64), np.nonzero(valid)[0]] = 1.0
        cores.append(dict(es=remap(es), ew=ew, mT=mT))

    # (f,h)-major permutation of the 780-wide hidden layout
    perm = np.arange(780).reshape(H, F).T.reshape(-1)  # fh -> natural index
    gat_W = np.asarray(inputs["gat_W"], np.float32)
    a_src = np.asarray(inputs["gat_a_src"], np.float32)
    a_dst = np.asarray(inputs["gat_a_dst"], np.float32)
    was = np.zeros((HID, H), np.float32)
    wad = np.zeros((HID, H), np.float32)
    for h in range(H):
        was[h * F : (h + 1) * F, h] = a_src[h]
        wad[h * F : (h + 1) * F, h] = a_dst[h]
    wpack = np.zeros((78, 1024), np.float32)
    wpack[:, 0:HID] = gat_W[:, perm]
    wpack[:, HID : HID + 10] = gat_W @ was
    wpack[:, HID + 10 : HID + 20] = gat_W @ wad
    gatb_fh = np.asarray(inputs["gat_b"], np.float32)[perm].reshape(1, HID)

    gcn_W_pad = np.zeros((896, HID), np.float32)
    gcn_W_pad[:HID] = np.asarray(inputs["gcn_W"], np.float32)[perm, :]
    fcg1_W_pad = np.zeros((896, 1500), np.float32)
    fcg1_W_pad[:HID] = np.asarray(inputs["fcg1_W"], np.float32)
    fcg2_W_pad = np.zeros((1536, 128), np.float32)
    fcg2_W_pad[:1500] = np.asarray(inputs["fcg2_W"], np.float32)

    fxt_W = np.asarray(inputs["fxt_W"], np.float32)
    fxt_Wp = np.zeros((KPAD, 128), np.float32)
    fxt_Wp.reshape(NF, 1024, 128)[:, :CONV_OUT] = fxt_W.reshape(NF, CONV_OUT, 128)

    cW = np.asarray(inputs["cW"], np.float32)
    cwt = np.ascontiguousarray(cW.transpose(2, 1, 0))  # [8, 128, 32]

    gbase = np.array([batch[c * NPC] for c in range(NCORES)], np.int64)
    span = np.array(
        [batch[min(c * NPC + NPC, N) - 1] - gbase[c] + 1 for c in range(NCORES)]
    )
    assert span.max() <= GSLOT, span.max()
    Cc_all = []
    for c in range(NCORES):
        Cmat = np.zeros((NCORES * GSLOT, MY_G), np.float32)
        for r in range(NCORES):
            for slot in range(GSLOT):
                g = gbase[r] + slot
                col = g - c * BPC
                if 0 <= col < BPC and g < B:
                    Cmat[r * GSLOT + slot, col] = 1.0
        Cc_all.append(Cmat)

    meta = dict(tpw=tpw, ntile=ntile, epc=epc)

    per_core = []
    for c in range(NCORES):
        ed_ = cores[c]
        bw = np.full(NPAD, -1000.0, np.float32)
        bw[:NPC] = (batch[c * NPC : (c + 1) * NPC] - gbase[c]).astype(np.float32)
        batchw = bw.reshape(NWIN, 128).T.copy()

        # one-hot token matrix [26, TOKPAD]: 5 groups x (5 graphs x 1000 + 120 pad)
        oh = np.zeros((VOC, TOKPAD), np.float32)
        tg = target[c * BPC : (c + 1) * BPC].reshape(5, 5 * SEQ)
        colbase = np.arange(5)[:, None] * 5120 + np.arange(5 * SEQ)[None, :]
        oh[tg.reshape(-1), colbase.reshape(-1)] = 1.0

        dstw = ed_["ew"].reshape(ntile, 128).T.copy()

        xTc = np.zeros((78, NPAD), np.float32)
        xTc[:, :NPC] = x[c * NPC : (c + 1) * NPC].T

        d = {
            "xTc": xTc.astype(BF),
            "wpack": wpack.astype(BF),
            "src16": _wrap16(ed_["es"], epc),
            "maskT": ed_["mT"].astype(BF),
            "dstw": dstw.astype(BF),
            "batchw": batchw,
            "Cc": Cc_all[c],
            "gatb": gatb_fh,
            "gcnw": gcn_W_pad.astype(BF),
            "gcn_b": np.asarray(inputs["gcn_b"], np.float32).reshape(1, HID),
            "fcg1_Wp": fcg1_W_pad,
            "fcg1_b": np.asarray(inputs["fcg1_b"], np.float32).reshape(1, 1500),
            "fcg2_Wp": fcg2_W_pad,
            "fcg2_b": np.asarray(inputs["fcg2_b"], np.float32).reshape(1, 128),
            "onehot": oh.astype(BF),
            "emb_bf": np.asarray(inputs["emb"], np.float32).astype(BF),
            "cwt_bf": cwt.astype(BF),
            "cb": np.asarray(inputs["cb"], np.float32).reshape(NF, 1),
            "fxt_Wp": fxt_Wp.astype(BF),
            "fxt_b": np.asarray(inputs["fxt_b"], np.float32).reshape(1, 128),
            "f1_W": np.asarray(inputs["f1_W"], np.float32),
            "f1_b": np.asarray(inputs["f1_b"], np.float32).reshape(1, 1024),
            "f2_W": np.asarray(inputs["f2_W"], np.float32),
            "f2_b": np.asarray(inputs["f2_b"], np.float32).reshape(1, 512),
            "f3_W": np.asarray(inputs["f3_W"], np.float32),
            "f3_b": np.asarray(inputs["f3_b"], np.float32).reshape(1, 256),
            "f4_W": np.asarray(inputs["f4_W"], np.float32),
            "f4_b": np.asarray(inputs["f4_b"], np.float32).reshape(1, 128),
            "o_W": np.asarray(inputs["o_W"], np.float32),
            "o_b": np.asarray(inputs["o_b"], np.float32).reshape(1, 1),
        }
        per_core.append(d)
    return per_core, meta


# ---------------------------------------------------------------- device build

_CACHE = {}


def build_bass(meta):
    key = (meta["tpw"], meta["ntile"], os.environ.get("KDEBUG", "0"))
    if key in _CACHE:
        return _CACHE[key]

    tpw, ntile, epc = meta["tpw"], meta["ntile"], meta["epc"]
    nchunk = -(-ntile // 16)

    nc = bacc.Bacc(
        "TRN2",
        target_bir_lowering=False,
        debug=False,
        num_devices=NCORES,
        num_swdge_queues=2,
    )

    def inp(name, shape, dt=F32):
        return nc.dram_tensor(name, list(shape), dt, kind="ExternalInput")

    xTc = inp("xTc", (78, NPAD), BF16)
    wpack = inp("wpack", (78, 1024), BF16)
    src16 = inp("src16", (128, epc // 16), I16)
    maskTd = inp("maskT", (128, epc), BF16)
    dstw = inp("dstw", (128, ntile), BF16)
    batchw = inp("batchw", (128, NWIN))
    Cc = inp("Cc", (NCORES * GSLOT, MY_G))
    gatb = inp("gatb", (1, HID))
    gcnw = inp("gcnw", (896, HID), BF16)
    gcn_b = inp("gcn_b", (1, HID))
    fcg1_Wp = inp("fcg1_Wp", (896, 1500))
    fcg1_b = inp("fcg1_b", (1, 1500))
    fcg2_Wp = inp("fcg2_Wp", (1536, 128))
    fcg2_b = inp("fcg2_b", (1, 128))
    onehot = inp("onehot", (VOC, TOKPAD), BF16)
    emb_bf = inp("emb_bf", (VOC, EMB), BF16)
    cwt_bf = inp("cwt_bf", (KS, EMB, NF), BF16)
    cb = inp("cb", (NF, 1))
    fxt_Wp = inp("fxt_Wp", (KPAD, 128), BF16)
    fxt_b = inp("fxt_b", (1, 128))
    f1_W = inp("f1_W", (256, 1024))
    f1_b = inp("f1_b", (1, 1024))
    f2_W = inp("f2_W", (1024, 512))
    f2_b = inp("f2_b", (1, 512))
    f3_W = inp("f3_W", (512, 256))
    f3_b = inp("f3_b", (1, 256))
    f4_W = inp("f4_W", (256, 128))
    f4_b = inp("f4_b", (1, 128))
    o_W = inp("o_W", (128, 1))
    o_b = inp("o_b", (1, 1))
    out_d = nc.dram_tensor("out", [MY_G, 1], F32, kind="ExternalOutput")
    KDEBUG = bool(int(os.environ.get("KDEBUG", "0")))
    if KDEBUG:
        out_h = nc.dram_tensor("out_h", [256, RBF], BF16, kind="ExternalOutput")
        out_h2 = nc.dram_tensor("out_h2", [256, RBF], BF16, kind="ExternalOutput")
        out_pool = nc.dram_tensor("out_pool", [NCORES * GSLOT, HID], F32, kind="ExternalOutput")
        out_dinv = nc.dram_tensor("out_dinv", [128, NWIN], F32, kind="ExternalOutput")
        out_xt = nc.dram_tensor("out_xt", [MY_G, 128], F32, kind="ExternalOutput")

    hinA = nc.dram_tensor("hinA", [HALFR, RBF], BF16)
    hinB = nc.dram_tensor("hinB", [HALFR, RBF], BF16)
    htabG = nc.dram_tensor("htabG", [2 * SEG, RBF], BF16, addr_space="Shared")
    aginA = nc.dram_tensor("aginA", [HALFR, RBF], BF16)
    aginB = nc.dram_tensor("aginB", [HALFR, RBF], BF16)
    htab2G = nc.dram_tensor("htab2G", [2 * SEG, RBF], BF16, addr_space="Shared")
    poolin = nc.dram_tensor("poolin", [GSLOT, HID], F32)
    poolall = nc.dram_tensor("poolall", [NCORES * GSLOT, HID], F32, addr_space="Shared")

    RG = [list(range(NCORES))]

    with tile.TileContext(nc) as tc:
        import contextlib

        ctx = contextlib.ExitStack()
        with ctx:
            pers = ctx.enter_context(tc.tile_pool(name="pers", bufs=1))

            # consts
            iota_i = pers.tile([128, 128], I32)
            nc.gpsimd.iota(iota_i[:], pattern=[[1, 128]], base=0, channel_multiplier=0)
            iota_f = pers.tile([128, 128], F32)
            nc.vector.tensor_copy(iota_f[:], iota_i[:])
            iota_bf = pers.tile([128, 1, 128], BF16)
            nc.vector.tensor_copy(iota_bf[:, 0, :], iota_i[:])
            ident = pers.tile([128, 128], F32)
            make_identity(nc, ident[:])
            ident_bf = pers.tile([128, 128], BF16)
            nc.vector.tensor_copy(ident_bf[:], ident[:])
            ones1 = pers.tile([1, 128], F32)
            nc.gpsimd.memset(ones1[:], 1.0)

            # residents
            dstw_t = pers.tile([128, ntile], BF16)
            nc.sync.dma_start(dstw_t[:], dstw[:, :])
            batchw_t = pers.tile([128, NWIN], F32)
            nc.sync.dma_start(batchw_t[:], batchw[:, :])
            src_t = pers.tile([128, epc // 16], I16)
            nc.sync.dma_start(src_t[:], src16[:, :])

            dinv_all = pers.tile([128, NWIN], F32)
            adw_all = pers.tile([128, NWIN, 10], BF16)
            asad_all = pers.tile([128, NWIN, 10], F32)
            xt_sb = pers.tile([MY_G, 128], F32)
            nc.gpsimd.memset(xt_sb[:], 0.0)
            gcnw_sb = pers.tile([128, 7, HID], BF16)
            nc.sync.dma_start(
                gcnw_sb[:], gcnw.ap().rearrange("(c p) f -> p c f", p=128)
            )

            # ---- phase 1: own h rows (f,h)-major; AllGather table in halves ----
            with (
                tc.tile_pool(name="p1", bufs=1) as p1,
                tc.tile_pool(name="p1h", bufs=3) as p1h,
                tc.tile_pool(name="ps1", bufs=1, space="PSUM") as ps1,
                # protein pools (shared scope so it can fill AllGather #1 time)
                tc.tile_pool(name="pp", bufs=1) as pp,
                tc.tile_pool(name="ppg", bufs=2) as ppg,
                tc.tile_pool(name="ppw", bufs=2) as ppw,
                tc.tile_pool(name="psE", bufs=1, space="PSUM") as psE,
                tc.tile_pool(name="psC", bufs=2, space="PSUM") as psC,
                tc.tile_pool(name="psTr", bufs=1, space="PSUM") as psTr,
                tc.tile_pool(name="psX", bufs=1, space="PSUM") as psX,
            ):
                xT_sb = p1.tile([78, NPAD], BF16)
                nc.sync.dma_start(xT_sb[:], xTc[:, :])
                wp_sb = p1.tile([78, 1024], BF16)
                nc.sync.dma_start(wp_sb[:], wpack[:, :])

                for w in range(NWIN):
                    hp = ps1.tile([128, 1024], F32, space="PSUM", tag="hp")
                    for n0, nn in ((0, 512), (512, 288)):
                        nc.tensor.matmul(
                            hp[:, n0 : n0 + nn],
                            lhsT=xT_sb[:, w * 128 : (w + 1) * 128],
                            rhs=wp_sb[:, n0 : n0 + nn],
                            start=True,
                            stop=True,
                        )
                    hrow = p1h.tile([128, RBF], BF16, tag="hrow")
                    nc.vector.tensor_copy(hrow[:, 0:HID], hp[:, 0:HID])
                    nc.vector.tensor_copy(
                        hrow[:, 780:800].bitcast(F32), hp[:, 780:790]
                    )
                    nc.gpsimd.memset(hrow[:, 800:801], 1.0)
                    nc.gpsimd.memset(hrow[:, 801:RBF], 0.0)
                    nc.vector.tensor_tensor(
                        asad_all[:, w, :],
                        hrow[:, 780:800].bitcast(F32),
                        hp[:, 790:800],
                        op=OP.add,
                    )
                    nc.vector.tensor_copy(adw_all[:, w, :], hp[:, 790:800])
                    if w < HALFW:
                        nc.sync.dma_start(
                            hinA.ap()[w * 128 : (w + 1) * 128, :], hrow[:]
                        )
                    else:
                        nc.sync.dma_start(
                            hinB.ap()[(w - HALFW) * 128 : (w - HALFW + 1) * 128, :],
                            hrow[:],
                        )
                    if w == HALFW - 1:
                        nc.gpsimd.collective_compute(
                            "AllGather", OP.bypass, replica_groups=RG,
                            ins=[hinA.ap().opt()],
                            outs=[htabG.ap()[0:SEG, :].opt()],
                        )
                    if w == NWIN - 1:
                        nc.gpsimd.collective_compute(
                            "AllGather", OP.bypass, replica_groups=RG,
                            ins=[hinB.ap().opt()],
                            outs=[htabG.ap()[SEG : 2 * SEG, :].opt()],
                        )

                def bcast_bias(dram, width, name):
                    t = pers.tile([128, width], F32, tag=f"bc_{name}")
                    row = pers.tile([1, width], F32, tag=f"br_{name}")
                    nc.sync.dma_start(row[:], dram[0:1, :])
                    for n0 in range(0, width, 512):
                        nn = min(512, width - n0)
                        ps = psTr.tile([128, 512], F32, space="PSUM", tag="bcps")
                        nc.tensor.matmul(
                            ps[:, :nn], lhsT=ones1[:], rhs=row[:, n0 : n0 + nn],
                            start=True, stop=True,
                        )
                        nc.any.tensor_copy(t[:, n0 : n0 + nn], ps[:, :nn])
                    return t

                gatb_bc = bcast_bias(gatb, HID, "gatb")
                gcnb_bc = bcast_bias(gcn_b, HID, "gcnb")
                fcg1b_bc = bcast_bias(fcg1_b, 1500, "fcg1b")
                fcg2b_bc = bcast_bias(fcg2_b, 128, "fcg2b")
                fxtb_bc = bcast_bias(fxt_b, 128, "fxtb")
                f1b_bc = bcast_bias(f1_b, 1024, "f1b")
                f2b_bc = bcast_bias(f2_b, 512, "f2b")
                f3b_bc = bcast_bias(f3_b, 256, "f3b")
                f4b_bc = bcast_bias(f4_b, 128, "f4b")
                ob_bc = bcast_bias(o_b, 1, "ob")


                # ---- protein branch (fills the AllGather #1 window) ----
                emb_sb = pp.tile([VOC, EMB], BF16)
                nc.sync.dma_start(emb_sb[:], emb_bf.ap()[:, :])
                cwt_sb = pp.tile([128, KS, NF], BF16)
                nc.sync.dma_start(cwt_sb[:], cwt_bf.ap().rearrange("k p o -> p k o"))
                cb_sb = pp.tile([NF, 1], F32)
                nc.sync.dma_start(cb_sb[:], cb.ap()[:, :])
                cT_all = pp.tile([128, 8, NF, BPC], BF16)

                for grp in range(5):
                    oh = ppg.tile([VOC, 5120], BF16, tag="oh")
                    nc.sync.dma_start(
                        oh[:], onehot.ap()[:, grp * 5120 : (grp + 1) * 5120]
                    )
                    et5 = ppg.tile([128, 5120], BF16, tag="et5")
                    for i in range(10):
                        eps_ = psE.tile([128, 512], F32, space="PSUM", tag="embps")
                        nc.tensor.matmul(
                            eps_[:], lhsT=emb_sb[:],
                            rhs=oh[:, i * 512 : (i + 1) * 512],
                            start=True, stop=True,
                        )
                        nc.any.tensor_copy(et5[:, i * 512 : (i + 1) * 512], eps_[:])
                    for bl in range(5):
                        b = grp * 5 + bl
                        boff = bl * 1000
                        csb = pp.tile([NF, 1024], F32, tag="csb")
                        for p0 in (0, 512):
                            cps = psC.tile([NF, 512], F32, space="PSUM", tag="cps")
                            for k in range(KS):
                                nc.tensor.matmul(
                                    cps[:, 0:512],
                                    lhsT=cwt_sb[:, k, :],
                                    rhs=et5[:, boff + k + p0 : boff + k + p0 + 512],
                                    start=(k == 0),
                                    stop=(k == KS - 1),
                                )
                            nc.scalar.activation(
                                csb[:, p0 : p0 + 512], cps[:, 0:512],
                                AF.Identity, bias=cb_sb[:, 0:1],
                            )
                        for pc in range(8):
                            trc = psTr.tile([128, 128], F32, space="PSUM", tag="trc")
                            nc.tensor.transpose(
                                trc[:, 0:NF],
                                csb[:, pc * 128 : (pc + 1) * 128],
                                ident[0:NF, 0:NF],
                            )
                            nc.any.tensor_copy(cT_all[:, pc, :, b], trc[:, 0:NF])

                xtps = psX.tile([MY_G, 128], F32, space="PSUM", tag="xtps")
                for sc in range(16):
                    wpt = ppw.tile([128, 16, 128], BF16, tag="wpt")
                    nc.sync.dma_start(
                        wpt[:],
                        fxt_Wp.ap()[sc * 2048 : (sc + 1) * 2048, :].rearrange(
                            "(c p) j -> p c j", p=128
                        ),
                    )
                    for sub in range(16):
                        q = sc * 16 + sub
                        o, t8 = q // 8, q % 8
                        nc.tensor.matmul(
                            xtps[0:BPC, :],
                            lhsT=cT_all[:, t8, o, :],
                            rhs=wpt[:, sub, :],
                            start=(q == 0),
                            stop=(q == 255),
                        )
                nc.vector.tensor_tensor(
                    xt_sb[0:BPC, :], xtps[0:BPC, :], fxtb_bc[0:BPC, :], op=OP.add
                )

            # shared edge-phase machinery -----------------------------------
            def edge_phase(table, gat, epilogue, extra_psum):
                with (
                    tc.tile_pool(name="msgp", bufs=3) as msgp,
                    tc.tile_pool(name="smallp", bufs=2) as smallp,
                    tc.tile_pool(name="maskp", bufs=3) as maskp,
                    tc.tile_pool(name="epip", bufs=2) as epip,
                    tc.tile_pool(name="hop", bufs=2) as hop,
                    tc.tile_pool(name="psA", bufs=2, space="PSUM") as psA,
                    tc.tile_pool(name="psS", bufs=1, space="PSUM") as psS,
                    tc.tile_pool(name="psD", bufs=1, space="PSUM") as psD,
                    extra_psum(tc) as psH,
                ):
                    aggp = None
                    hown = {}
                    rbsrc = (hinA, hinB) if gat else (aginA, aginB)
                    for c in range(nchunk):
                        T = min(16, ntile - c * 16)
                        msg = msgp.tile([128, 16, RBF], BF16, tag="msg")
                        nc.gpsimd.dma_gather(
                            msg[:, 0:T, :],
                            table.ap()[:, 0:RBF],
                            src_t[:, c * 128 : c * 128 + T * 8],
                            num_idxs=T * 128,
                            num_idxs_reg=T * 128,
                            elem_size=RBF,
                            elem_step=RBF,
                            single_packet=False,
                        )
                        maskall = maskp.tile([128, 16, 128], BF16, tag="maskall")
                        nc.vector.tensor_tensor(
                            maskall[:, 0:T, :],
                            dstw_t[:, c * 16 : c * 16 + T, None].to_broadcast(
                                [128, T, 128]
                            ),
                            iota_bf[:].to_broadcast([128, T, 128]),
                            op=OP.is_equal,
                        )
                        if gat:
                            sall = smallp.tile([128, 16, 10], F32, tag="sall")
                            sl2 = smallp.tile([128, 16, 10], F32, tag="sl2")
                            mTc = maskp.tile([128, 16, 128], BF16, tag="mTc")
                            nc.sync.dma_start(
                                mTc[:, 0:T, :],
                                maskTd.ap()[:, c * 2048 : c * 2048 + T * 128]
                                .rearrange("p (t e) -> p t e", e=128),
                            )
                            adx = psD.tile([128, 512], F32, space="PSUM", tag="adx")
                            for j in range(T):
                                g = c * 16 + j
                                nc.tensor.matmul(
                                    adx[:, j * 10 : j * 10 + 10],
                                    lhsT=mTc[:, j, :],
                                    rhs=adw_all[:, g // tpw, :],
                                    start=True,
                                    stop=True,
                                )
                            nc.vector.tensor_tensor(
                                sall[:, 0:T, :],
                                msg[:, 0:T, 780:800].bitcast(F32),
                                adx[:, 0 : T * 10].rearrange("p (a b) -> p a b", b=10),
                                op=OP.add,
                            )
                            nc.vector.tensor_scalar_mul(
                                sl2[:, 0:T, :], sall[:, 0:T, :], 0.2
                            )
                            nc.vector.tensor_tensor(
                                sl2[:, 0:T, :], sall[:, 0:T, :], sl2[:, 0:T, :],
                                op=OP.max,
                            )
                            nc.scalar.activation(
                                msg[:, 0:T, 780:790], sl2[:, 0:T, :], AF.Exp
                            )
                            nc.vector.tensor_tensor(
                                msg[:, 0:T, 0:HID].rearrange(
                                    "p c (f h) -> p c f h", h=H
                                ),
                                msg[:, 0:T, 0:HID].rearrange(
                                    "p c (f h) -> p c f h", h=H
                                ),
                                msg[:, 0:T, 780:790][:, :, None, :].to_broadcast(
                                    [128, T, F, H]
                                ),
                                op=OP.mult,
                            )
                        # pass 2: scatter matmuls
                        n_hi = 801 if gat else HID
                        for j in range(T):
                            g = c * 16 + j
                            w, r = divmod(g, tpw)
                            if r == 0:
                                aggp = psA.tile(
                                    [128, 1024], F32, space="PSUM", tag="aggp"
                                )
                                ht = hop.tile([128, RBF], BF16, tag="hown")
                                rb = rbsrc[0] if w < HALFW else rbsrc[1]
                                ro = (w % HALFW) * 128
                                nc.sync.dma_start(ht[:], rb.ap()[ro : ro + 128, :])
                                hown[w] = ht
                            for n0, nn in ((0, 512), (512, n_hi - 512)):
                                nc.tensor.matmul(
                                    aggp[:, n0 : n0 + nn],
                                    lhsT=maskall[:, j, :],
                                    rhs=msg[:, j, n0 : n0 + nn],
                                    start=(r == 0),
                                    stop=(r == tpw - 1),
                                )
                            if r == tpw - 1:
                                epilogue(w, aggp, epip, hown.pop(w), psH, psS)

            # ---- phase 2: GAT edge phase (h2 + AllGather #2 interleaved) ----
            def gat_epilogue(w, aggp, epip, hown, psH, psS):
                exs1 = epip.tile([128, 10], F32, tag="exs1")
                nc.vector.tensor_scalar_mul(exs1[:], asad_all[:, w, :], 0.2)
                nc.vector.tensor_tensor(
                    exs1[:], asad_all[:, w, :], exs1[:], op=OP.max
                )
                exs2 = epip.tile([128, 10], F32, tag="exs2")
                nc.scalar.activation(exs2[:], exs1[:], AF.Exp)
                rec = epip.tile([128, 10], F32, tag="rec")
                nc.vector.tensor_tensor(rec[:], aggp[:, 780:790], exs2[:], op=OP.add)
                rcp = epip.tile([128, 10], F32, tag="rcp")
                nc.vector.reciprocal(rcp[:], rec[:])
                dsq = epip.tile([128, 1], F32, tag="dsq")
                nc.scalar.activation(dsq[:], aggp[:, 800:801], AF.Sqrt, bias=1.0)
                nc.vector.reciprocal(dinv_all[:, w : w + 1], dsq[:])
                selfm = epip.tile([128, HID], F32, tag="selfm")
                nc.vector.tensor_tensor(
                    selfm[:].rearrange("p (f h) -> p f h", h=H),
                    hown[:, 0:HID].rearrange("p (f h) -> p f h", h=H),
                    exs2[:, None, :].to_broadcast([128, F, H]),
                    op=OP.mult,
                )
                x1s = epip.tile([128, HID], F32, tag="x1s")
                nc.vector.tensor_tensor(x1s[:], aggp[:, 0:HID], selfm[:], op=OP.add)
                nc.vector.tensor_tensor(
                    x1s[:].rearrange("p (f h) -> p f h", h=H),
                    x1s[:].rearrange("p (f h) -> p f h", h=H),
                    rcp[:, None, :].to_broadcast([128, F, H]),
                    op=OP.mult,
                )
                nc.vector.tensor_tensor(x1s[:], x1s[:], gatb_bc[:, 0:HID], op=OP.add)
                x1w = epip.tile([128, HID], BF16, tag="x1w")
                nc.scalar.activation(x1w[:], x1s[:], AF.Relu)
                # h2 = relu(x1) @ gcn_W, scaled by dinv, written to agin
                x1T = epip.tile([128, 7, 128], BF16, tag="x1T")
                for kc in range(7):
                    sz = 128 if kc < 6 else 12
                    trp = psS.tile([128, 512], BF16, space="PSUM", tag="trT")
                    nc.tensor.transpose(
                        trp[0:sz, 0:128], x1w[:, kc * 128 : kc * 128 + sz], ident_bf[:]
                    )
                    nc.any.tensor_copy(x1T[0:sz, kc, :], trp[0:sz, 0:128])
                h2ps = psH.tile([128, 1024], F32, space="PSUM", tag="h2ps")
                for kc in range(7):
                    sz = 128 if kc < 6 else 12
                    for n0, nn in ((0, 512), (512, 268)):
                        nc.tensor.matmul(
                            h2ps[:, n0 : n0 + nn],
                            lhsT=x1T[0:sz, kc, :],
                            rhs=gcnw_sb[0:sz, kc, n0 : n0 + nn],
                            start=(kc == 0),
                            stop=(kc == 6),
                        )
                h2s = epip.tile([128, RBF], BF16, tag="h2s")
                nc.scalar.activation(
                    h2s[:, 0:HID], h2ps[:, 0:HID], AF.Identity,
                    scale=dinv_all[:, w : w + 1],
                )
                if w < HALFW:
                    nc.sync.dma_start(
                        aginA.ap()[w * 128 : (w + 1) * 128, :], h2s[:]
                    )
                else:
                    nc.sync.dma_start(
                        aginB.ap()[(w - HALFW) * 128 : (w - HALFW + 1) * 128, :],
                        h2s[:],
                    )
                if w == HALFW - 1:
                    nc.gpsimd.collective_compute(
                        "AllGather", OP.bypass, replica_groups=RG,
                        ins=[aginA.ap().opt()],
                        outs=[htab2G.ap()[0:SEG, :].opt()],
                    )
                if w == NWIN - 1:
                    nc.gpsimd.collective_compute(
                        "AllGather", OP.bypass, replica_groups=RG,
                        ins=[aginB.ap().opt()],
                        outs=[htab2G.ap()[SEG : 2 * SEG, :].opt()],
                    )

            def psH_gat(tc):
                return tc.tile_pool(name="psH", bufs=1, space="PSUM")

            edge_phase(htabG, True, gat_epilogue, psH_gat)

            # ---- phase 3: GCN edge phase + pooling; phase 4: head ----
            with tc.tile_pool(name="psP", bufs=1, space="PSUM") as psP:
                poolps = psP.tile([GSLOT, 1024], F32, space="PSUM", tag="poolps")

                def gcn_epilogue(w, aggp, epip, h2own, psH, psS):
                    x2s = epip.tile([128, HID], F32, tag="x2s")
                    nc.vector.tensor_tensor(
                        x2s[:], aggp[:, 0:HID], h2own[:, 0:HID], op=OP.add
                    )
                    x2d = epip.tile([128, HID], F32, tag="x2d")
                    nc.scalar.activation(
                        x2d[:], x2s[:], AF.Identity, scale=dinv_all[:, w : w + 1]
                    )
                    nc.vector.tensor_tensor(
                        x2d[:], x2d[:], gcnb_bc[:, 0:HID], op=OP.add
                    )
                    x2w = epip.tile([128, HID], F32, tag="x2w")
                    nc.scalar.activation(x2w[:], x2d[:], AF.Relu)
                    ph = epip.tile([128, GSLOT], F32, tag="poolhot")
                    nc.vector.tensor_tensor(
                        ph[:],
                        batchw_t[:, w : w + 1].to_broadcast([128, GSLOT]),
                        iota_f[:, 0:GSLOT],
                        op=OP.is_equal,
                    )
                    for n0, nn in ((0, 512), (512, 268)):
                        nc.tensor.matmul(
                            poolps[:, n0 : n0 + nn],
                            lhsT=ph[:],
                            rhs=x2w[:, n0 : n0 + nn],
                            start=(w == 0),
                            stop=(w == NWIN - 1),
                        )

                def psH_gcn(tc):
                    return tc.tile_pool(name="psH2", bufs=1, space="PSUM")

                edge_phase(htab2G, False, gcn_epilogue, psH_gcn)
                poolsb = pers.tile([GSLOT, HID], F32)
                nc.any.tensor_copy(poolsb[:], poolps[:, 0:HID])

            with (
                tc.tile_pool(name="p5", bufs=1) as p5,
                tc.tile_pool(name="p5w", bufs=2) as p5w,
                tc.tile_pool(name="ps5", bufs=2, space="PSUM") as ps5,
                tc.tile_pool(name="ps5t", bufs=2, space="PSUM") as ps5t,
            ):
                nc.sync.dma_start(poolin.ap()[:, :], poolsb[:])
                nc.gpsimd.collective_compute(
                    "AllGather", OP.bypass, replica_groups=RG,
                    ins=[poolin.ap().opt()],
                    outs=[poolall.ap().opt()],
                )
                Cc_sb = p5.tile([128, 4, MY_G], F32)
                nc.sync.dma_start(
                    Cc_sb[:], Cc.ap().rearrange("(c p) g -> p c g", p=128)
                )
                pall = p5.tile([128, 4, HID], F32)
                nc.sync.dma_start(
                    pall[:], poolall.ap().rearrange("(c p) f -> p c f", p=128)
                )
                xgps = ps5.tile([MY_G, 1024], F32, space="PSUM", tag="mlp_ps")
                for kc in range(4):
                    for n0, nn in ((0, 512), (512, 268)):
                        nc.tensor.matmul(
                            xgps[:, n0 : n0 + nn],
                            lhsT=Cc_sb[:, kc, :],
                            rhs=pall[:, kc, n0 : n0 + nn],
                            start=(kc == 0),
                            stop=(kc == 3),
                        )
                xg = p5.tile([MY_G, HID], F32, tag="act0")
                nc.any.tensor_copy(xg[:], xgps[:, 0:HID])

                def dense(x_sb, k_real, w_dram, w_rows, n_out, b_bc, relu, tag):
                    nkc = (k_real + 127) // 128
                    xT_t = p5.tile([128, nkc, MY_G], F32, tag="xT5")
                    for kc in range(nkc):
                        sz = min(128, k_real - kc * 128)
                        trp = ps5t.tile([128, MY_G], F32, space="PSUM", tag="tr5")
                        nc.tensor.transpose(
                            trp[0:sz, :],
                            x_sb[:, kc * 128 : kc * 128 + sz],
                            ident[0:MY_G, 0:MY_G],
                        )
                        nc.any.tensor_copy(xT_t[0:sz, kc, :], trp[0:sz, :])
                    w_sb = p5w.tile([128, w_rows // 128, n_out], F32, tag="w5")
                    nc.sync.dma_start(
                        w_sb[:], w_dram.ap().rearrange("(c p) f -> p c f", p=128)
                    )
                    yps = ps5.tile([MY_G, 1536], F32, space="PSUM", tag="mlp_ps")
                    for n0 in range(0, n_out, 512):
                        nn = min(512, n_out - n0)
                        for kc in range(nkc):
                            sz = min(128, k_real - kc * 128)
                            nc.tensor.matmul(
                                yps[:, n0 : n0 + nn],
                                lhsT=xT_t[0:sz, kc, :],
                                rhs=w_sb[0:sz, kc, n0 : n0 + nn],
                                start=(kc == 0),
                                stop=(kc == nkc - 1),
                            )
                    yf = p5.tile([MY_G, n_out], F32, tag="y5f")
                    nc.vector.tensor_tensor(
                        yf[:], yps[:, 0:n_out], b_bc[0:MY_G, 0:n_out], op=OP.add
                    )
                    y = p5.tile([MY_G, n_out], F32, tag="y5")
                    nc.scalar.activation(
                        y[:], yf[:], AF.Relu if relu else AF.Identity
                    )
                    return y, yf

                y1, _ = dense(xg, HID, fcg1_Wp, 896, 1500, fcg1b_bc, True, "fcg1")
                xgo, _ = dense(y1, 1500, fcg2_Wp, 1536, 128, fcg2b_bc, False, "fcg2")
                xc = p5.tile([MY_G, 256], F32, tag="xc")
                nc.any.tensor_copy(xc[:, 0:128], xgo[:])
                nc.any.tensor_copy(xc[:, 128:256], xt_sb[:])
                a1, _ = dense(xc, 256, f1_W, 256, 1024, f1b_bc, True, "f1")
                a2, _ = dense(a1, 1024, f2_W, 1024, 512, f2b_bc, True, "f2")
                a3, _ = dense(a2, 512, f3_W, 512, 256, f3b_bc, True, "f3")
                a4, _ = dense(a3, 256, f4_W, 256, 128, f4b_bc, True, "f4")
                _, yof = dense(a4, 128, o_W, 128, 1, ob_bc, False, "o")
                nc.sync.dma_start(out_d.ap()[:, :], yof[:])
                if KDEBUG:
                    dbg = p5.tile([128, 2, RBF], BF16, tag="dbg")
                    nc.sync.dma_start(
                        dbg[:], htabG.ap()[0:256, :].rearrange("(c p) f -> p c f", p=128)
                    )
                    nc.sync.dma_start(
                        out_h.ap().rearrange("(c p) f -> p c f", p=128), dbg[:]
                    )
                    dbg2 = p5.tile([128, 2, RBF], BF16, tag="dbg2")
                    nc.sync.dma_start(
                        dbg2[:], htab2G.ap()[0:256, :].rearrange("(c p) f -> p c f", p=128)
                    )
                    nc.sync.dma_start(
                        out_h2.ap().rearrange("(c p) f -> p c f", p=128), dbg2[:]
                    )
                    dbg3 = p5.tile([128, 4, HID], F32, tag="dbg3")
                    nc.sync.dma_start(
                        dbg3[:], poolall.ap().rearrange("(c p) f -> p c f", p=128)
                    )
                    nc.sync.dma_start(
                        out_pool.ap().rearrange("(c p) f -> p c f", p=128), dbg3[:]
                    )
                    nc.sync.dma_start(out_dinv.ap()[:, :], dinv_all[:].rearrange("p a -> p a"))
                    nc.sync.dma_start(out_xt.ap()[:, :], xt_sb[:])

    nc.compile()
    _CACHE[key] = nc
    return nc


# ---------------------------------------------------------------- entry point


def _ensure_ntff_hook():
    """Install antenv.axon_hooks + register the ctypes NTFF hook if the image
    lacks them (profiling only; failures are non-fatal)."""
    import types

    try:
        import antenv.axon_hooks  # noqa: F401

        if antenv.axon_hooks.get_axon_ntff_profile_hook() is not None:
            return
    except ImportError:
        import antenv

        mod = types.ModuleType("antenv.axon_hooks")
        mod._hook = None

        def set_axon_ntff_profile_hook(h, _m=mod):
            _m._hook = h

        def get_axon_ntff_profile_hook(_m=mod):
            return _m._hook

        mod.set_axon_ntff_profile_hook = set_axon_ntff_profile_hook
        mod.get_axon_ntff_profile_hook = get_axon_ntff_profile_hook
        sys.modules["antenv.axon_hooks"] = mod
        antenv.axon_hooks = mod
    try:
        from antenv.axon_hooks import set_axon_ntff_profile_hook as _set
        from trn_agent_boot.trn_boot import _ntff_profile_via_ctypes

        hook = _ntff_profile_via_ctypes("/opt/axon/libaxon_pjrt.so")
        if hook is not None:
            _set(hook)
    except Exception:
        pass


def kernel(**inputs) -> np.ndarray:
    per_core, meta = host_prep(inputs)
    nc = build_bass(meta)
    in_maps = [{k: np.ascontiguousarray(v) for k, v in d.items()} for d in per_core]
    trace = bool(int(os.environ.get("KERNEL_TRACE", "0")))
    if trace:
        _ensure_ntff_hook()
    res = run_bass_kernel_spmd(nc, in_maps, core_ids=list(range(NCORES)), trace=trace)
    if trace and res.exec_time_ns is not None:
        print(f"HW exec time: {res.exec_time_ns} ns")
        kernel.last_exec_ns = res.exec_time_ns
    out = np.concatenate([res.results[c]["out"][:BPC] for c in range(NCORES)], 0)
    return out.astype(np.float32)


# revision 34
# speedup vs baseline: 1.5652x; 1.0344x over previous
"""Trainium2 Bass kernel for nn_EnhancedGATGCN (GAT -> GCN -> pool -> MLP, + protein conv branch).

Self-contained: host-side sharding prep + 8-core SPMD Bass/Tile device program.

Sharding strategy (v2):
  - Edges (self loops handled locally) sorted by dst, partitioned into 8
    contiguous dst ranges of 2500 nodes; scatter-add is core-local via one-hot
    matmuls on the TensorEngine with PSUM accumulation per 128-dst window.
  - Node feature tables (h for GAT, dinv-scaled h2 for GCN) are computed
    node-sharded in (f,h)-major feature layout, AllGathered as bf16 tables in
    HBM in two halves (windows 0-9 / 10-19) so the first half overlaps
    compute; per-edge messages are fetched with SWDGE dma_gather.
  - GCN input h2 = relu(x1) @ gcn_W is computed inside the GAT epilogue per
    window (no DRAM bounce), so AllGather #2 starts as soon as the GAT edge
    phase drains.
  - The protein branch (embedding via host-built one-hot matmul + conv + fxt)
    runs at the front, hidden under phase 1 + AllGather #1.
  - Dense tail is data-parallel over the 200-graph batch (25/core).
"""
import os
import sys

import numpy as np

sys.path.insert(0, "/opt/trn_rl_repo")

import ml_dtypes

import concourse.bacc as bacc
import concourse.bass as bass
import concourse.mybir as mybir
import concourse.tile as tile
from concourse.bass_utils import run_bass_kernel_spmd
from concourse.masks import make_identity

F32 = mybir.dt.float32
BF16 = mybir.dt.bfloat16
I16 = mybir.dt.int16
I32 = mybir.dt.int32
AF = mybir.ActivationFunctionType
OP = mybir.AluOpType
BF = ml_dtypes.bfloat16

N, E, B, H, F = 20000, 400000, 200, 10, 78
HID = H * F  # 780
SEQ, VOC, EMB, NF, KS = 1000, 26, 128, 32, 8
CONV_OUT = SEQ - KS + 1  # 993

NCORES = 8
NPC = N // NCORES  # 2500
NPAD = 2560
NWIN = NPAD // 128  # 20
HALFW = 10  # windows per table half
HALFR = HALFW * 128  # 1280 rows per core per half
SEG = NCORES * HALFR  # 10240 rows per table segment
RBF = 896  # bf16 cols per table row; bytes = 1792 (%256==0)
# GAT row: [0:780 h bf16 (f,h)-major | 780:800 a_s 10xf32 | 800 one | 801:896 zeros]
# during edge pass 780:790 is overwritten with exp(leaky(e)) per edge
GSLOT = 64
MY_G = 32
BPC = B // NCORES  # 25
TOKPAD = 25600  # 5 groups x 5120 (each: 5 graphs x 1000 tok + 120 pad)
KPAD = 32 * 1024


# ---------------------------------------------------------------- host prep


def _wrap16(idx, epc):
    a = np.zeros((128, epc // 16), np.int16)
    w = idx.reshape(epc // 16, 16).T
    a[:, :] = np.tile(w, (8, 1))
    return a


def host_prep(inputs):
    x = np.asarray(inputs["x"], np.float32)
    edge_index = np.asarray(inputs["edge_index"], np.int64)
    batch = np.asarray(inputs["batch"], np.int64)
    target = np.asarray(inputs["target"], np.int64)

    src = edge_index[0]
    dst = edge_index[1]
    order = np.argsort(dst, kind="stable")
    src, dst = src[order], dst[order]

    core_of = dst // NPC
    dst_local = dst - core_of * NPC
    win = dst_local // 128
    maxw = 0
    per_core_edges = []
    for c in range(NCORES):
        m = core_of == c
        s_c, dl_c, w_c = src[m], dst_local[m], win[m]
        per_core_edges.append((s_c, dl_c, w_c))
        maxw = max(maxw, int(np.bincount(w_c, minlength=NWIN).max()))
    tpw = -(-maxw // 128)
    ntile = NWIN * tpw
    epc = ntile * 128

    def remap(n):
        c = n // NPC
        l = n % NPC
        return np.where(l < HALFR, c * HALFR + l, SEG + c * HALFR + (l - HALFR))

    def remap4(n):
        c = n // NPC
        l = n % NPC
        s = np.minimum(l // 640, 3)
        return s * (NCORES * 640) + c * 640 + (l - s * 640)

    deg = np.bincount(dst, minlength=N).astype(np.float32) + 1.0
    dinv_host = 1.0 / np.sqrt(deg)

    cores = []
    for c in range(NCORES):
        s_c, dl_c, w_c = per_core_edges[c]
        es = np.zeros(epc, np.int64)
        ew = np.full(epc, -1000.0, np.float32)
        for w in range(NWIN):
            m = w_c == w
            k = int(m.sum())
            o = w * tpw * 128
            es[o : o + k] = s_c[m]
            ew[o : o + k] = (dl_c[m] - w * 128).astype(np.float32)
        mT = np.zeros((128, epc), np.float32)
        valid = ew >= 0
        mT[ew[valid].astype(np.int64), np.nonzero(valid)[0]] = 1.0
        cores.append(dict(es=remap(es), es4=remap4(es), ew=ew, mT=mT))

    # (f,h)-major permutation of the 780-wide hidden layout
    perm = np.arange(780).reshape(H, F).T.reshape(-1)  # fh -> natural index
    gat_W = np.asarray(inputs["gat_W"], np.float32)
    a_src = np.asarray(inputs["gat_a_src"], np.float32)
    a_dst = np.asarray(inputs["gat_a_dst"], np.float32)
    was = np.zeros((HID, H), np.float32)
    wad = np.zeros((HID, H), np.float32)
    for h in range(H):
        was[h * F : (h + 1) * F, h] = a_src[h]
        wad[h * F : (h + 1) * F, h] = a_dst[h]
    wpack = np.zeros((78, 1024), np.float32)
    wpack[:, 0:HID] = gat_W[:, perm]
    wpack[:, HID : HID + 10] = gat_W @ was
    wpack[:, HID + 10 : HID + 20] = gat_W @ wad
    gatb_fh = np.asarray(inputs["gat_b"], np.float32)[perm].reshape(1, HID)

    gcn_W_pad = np.zeros((896, HID), np.float32)
    gcn_W_pad[:HID] = np.asarray(inputs["gcn_W"], np.float32)[perm, :]
    fcg1_W_pad = np.zeros((896, 1500), np.float32)
    fcg1_W_pad[:HID] = np.asarray(inputs["fcg1_W"], np.float32)
    fcg2_W_pad = np.zeros((1536, 128), np.float32)
    fcg2_W_pad[:1500] = np.asarray(inputs["fcg2_W"], np.float32)

    fxt_W = np.asarray(inputs["fxt_W"], np.float32)
    fxt_Wp = np.zeros((KPAD, 128), np.float32)
    fxt_Wp.reshape(NF, 1024, 128)[:, :CONV_OUT] = fxt_W.reshape(NF, CONV_OUT, 128)

    cW = np.asarray(inputs["cW"], np.float32)
    cwt = np.ascontiguousarray(cW.transpose(2, 1, 0))  # [8, 128, 32]

    gbase = np.array([batch[c * NPC] for c in range(NCORES)], np.int64)
    span = np.array(
        [batch[min(c * NPC + NPC, N) - 1] - gbase[c] + 1 for c in range(NCORES)]
    )
    assert span.max() <= GSLOT, span.max()
    Cc_all = []
    for c in range(NCORES):
        Cmat = np.zeros((NCORES * GSLOT, MY_G), np.float32)
        for r in range(NCORES):
            for slot in range(GSLOT):
                g = gbase[r] + slot
                col = g - c * BPC
                if 0 <= col < BPC and g < B:
                    Cmat[r * GSLOT + slot, col] = 1.0
        Cc_all.append(Cmat)

    meta = dict(tpw=tpw, ntile=ntile, epc=epc)

    per_core = []
    for c in range(NCORES):
        ed_ = cores[c]
        bw = np.full(NPAD, -1000.0, np.float32)
        bw[:NPC] = (batch[c * NPC : (c + 1) * NPC] - gbase[c]).astype(np.float32)
        batchw = bw.reshape(NWIN, 128).T.copy()

        # one-hot token matrix [26, TOKPAD]: 5 groups x (5 graphs x 1000 + 120 pad)
        oh = np.zeros((VOC, TOKPAD), np.float32)
        tg = target[c * BPC : (c + 1) * BPC].reshape(5, 5 * SEQ)
        colbase = np.arange(5)[:, None] * 5120 + np.arange(5 * SEQ)[None, :]
        oh[tg.reshape(-1), colbase.reshape(-1)] = 1.0

        dstw = ed_["ew"].reshape(ntile, 128).T.copy()

        xTc = np.zeros((78, NPAD), np.float32)
        xTc[:, :NPC] = x[c * NPC : (c + 1) * NPC].T

        dv = np.ones(NPAD, np.float32)
        dv[:NPC] = dinv_host[c * NPC : (c + 1) * NPC]
        dinvw = dv.reshape(NWIN, 128).T.copy()

        d = {
            "xTc": xTc.astype(BF),
            "wpack": wpack.astype(BF),
            "src16": _wrap16(ed_["es"], epc),
            "src16b": _wrap16(ed_["es4"], epc),
            "maskT": ed_["mT"].astype(BF),
            "dstw": dstw.astype(BF),
            "batchw": batchw,
            "dinvw": dinvw,
            "Cc": Cc_all[c],
            "gatb": gatb_fh,
            "gcnw": gcn_W_pad.astype(BF),
            "gcn_b": np.asarray(inputs["gcn_b"], np.float32).reshape(1, HID),
            "fcg1_Wp": fcg1_W_pad,
            "fcg1_b": np.asarray(inputs["fcg1_b"], np.float32).reshape(1, 1500),
            "fcg2_Wp": fcg2_W_pad,
            "fcg2_b": np.asarray(inputs["fcg2_b"], np.float32).reshape(1, 128),
            "onehot": oh.astype(BF),
            "emb_bf": np.asarray(inputs["emb"], np.float32).astype(BF),
            "cwt_bf": cwt.astype(BF),
            "cb": np.asarray(inputs["cb"], np.float32).reshape(NF, 1),
            "fxt_Wp": fxt_Wp.astype(BF),
            "fxt_b": np.asarray(inputs["fxt_b"], np.float32).reshape(1, 128),
            "f1_W": np.asarray(inputs["f1_W"], np.float32),
            "f1_b": np.asarray(inputs["f1_b"], np.float32).reshape(1, 1024),
            "f2_W": np.asarray(inputs["f2_W"], np.float32),
            "f2_b": np.asarray(inputs["f2_b"], np.float32).reshape(1, 512),
            "f3_W": np.asarray(inputs["f3_W"], np.float32),
            "f3_b": np.asarray(inputs["f3_b"], np.float32).reshape(1, 256),
            "f4_W": np.asarray(inputs["f4_W"], np.float32),
            "f4_b": np.asarray(inputs["f4_b"], np.float32).reshape(1, 128),
            "o_W": np.asarray(inputs["o_W"], np.float32),
            "o_b": np.asarray(inputs["o_b"], np.float32).reshape(1, 1),
        }
        per_core.append(d)
    return per_core, meta


# ---------------------------------------------------------------- device build

_CACHE = {}


def build_bass(meta):
    key = (meta["tpw"], meta["ntile"], os.environ.get("KDEBUG", "0"))
    if key in _CACHE:
        return _CACHE[key]

    tpw, ntile, epc = meta["tpw"], meta["ntile"], meta["epc"]
    nchunk = -(-ntile // 16)

    nc = bacc.Bacc(
        "TRN2",
        target_bir_lowering=False,
        debug=False,
        num_devices=NCORES,
        num_swdge_queues=2,
    )

    def inp(name, shape, dt=F32):
        return nc.dram_tensor(name, list(shape), dt, kind="ExternalInput")

    xTc = inp("xTc", (78, NPAD), BF16)
    wpack = inp("wpack", (78, 1024), BF16)
    src16 = inp("src16", (128, epc // 16), I16)
    src16b = inp("src16b", (128, epc // 16), I16)
    dinvw = inp("dinvw", (128, NWIN))
    maskTd = inp("maskT", (128, epc), BF16)
    dstw = inp("dstw", (128, ntile), BF16)
    batchw = inp("batchw", (128, NWIN))
    Cc = inp("Cc", (NCORES * GSLOT, MY_G))
    gatb = inp("gatb", (1, HID))
    gcnw = inp("gcnw", (896, HID), BF16)
    gcn_b = inp("gcn_b", (1, HID))
    fcg1_Wp = inp("fcg1_Wp", (896, 1500))
    fcg1_b = inp("fcg1_b", (1, 1500))
    fcg2_Wp = inp("fcg2_Wp", (1536, 128))
    fcg2_b = inp("fcg2_b", (1, 128))
    onehot = inp("onehot", (VOC, TOKPAD), BF16)
    emb_bf = inp("emb_bf", (VOC, EMB), BF16)
    cwt_bf = inp("cwt_bf", (KS, EMB, NF), BF16)
    cb = inp("cb", (NF, 1))
    fxt_Wp = inp("fxt_Wp", (KPAD, 128), BF16)
    fxt_b = inp("fxt_b", (1, 128))
    f1_W = inp("f1_W", (256, 1024))
    f1_b = inp("f1_b", (1, 1024))
    f2_W = inp("f2_W", (1024, 512))
    f2_b = inp("f2_b", (1, 512))
    f3_W = inp("f3_W", (512, 256))
    f3_b = inp("f3_b", (1, 256))
    f4_W = inp("f4_W", (256, 128))
    f4_b = inp("f4_b", (1, 128))
    o_W = inp("o_W", (128, 1))
    o_b = inp("o_b", (1, 1))
    out_d = nc.dram_tensor("out", [MY_G, 1], F32, kind="ExternalOutput")
    KDEBUG = bool(int(os.environ.get("KDEBUG", "0")))
    if KDEBUG:
        out_h = nc.dram_tensor("out_h", [256, RBF], BF16, kind="ExternalOutput")
        out_h2 = nc.dram_tensor("out_h2", [256, RBF], BF16, kind="ExternalOutput")
        out_pool = nc.dram_tensor("out_pool", [NCORES * GSLOT, HID], F32, kind="ExternalOutput")
        out_dinv = nc.dram_tensor("out_dinv", [128, NWIN], F32, kind="ExternalOutput")
        out_xt = nc.dram_tensor("out_xt", [MY_G, 128], F32, kind="ExternalOutput")

    hinA = nc.dram_tensor("hinA", [HALFR, RBF], BF16)
    hinB = nc.dram_tensor("hinB", [HALFR, RBF], BF16)
    htabG = nc.dram_tensor("htabG", [2 * SEG, RBF], BF16, addr_space="Shared")
    agins = [
        nc.dram_tensor(f"agin{i}", [640, RBF], BF16) for i in range(4)
    ]
    htab2G = nc.dram_tensor("htab2G", [2 * SEG, RBF], BF16, addr_space="Shared")
    poolin = nc.dram_tensor("poolin", [GSLOT, HID], F32)
    poolall = nc.dram_tensor("poolall", [NCORES * GSLOT, HID], F32, addr_space="Shared")

    RG = [list(range(NCORES))]

    with tile.TileContext(nc) as tc:
        import contextlib

        ctx = contextlib.ExitStack()
        with ctx:
            pers = ctx.enter_context(tc.tile_pool(name="pers", bufs=1))

            # consts
            iota_i = pers.tile([128, 128], I32)
            nc.gpsimd.iota(iota_i[:], pattern=[[1, 128]], base=0, channel_multiplier=0)
            iota_f = pers.tile([128, 128], F32)
            nc.vector.tensor_copy(iota_f[:], iota_i[:])
            iota_bf = pers.tile([128, 1, 128], BF16)
            nc.vector.tensor_copy(iota_bf[:, 0, :], iota_i[:])
            ident = pers.tile([128, 128], F32)
            make_identity(nc, ident[:])
            ident_bf = pers.tile([128, 128], BF16)
            nc.vector.tensor_copy(ident_bf[:], ident[:])
            ones1 = pers.tile([1, 128], F32)
            nc.gpsimd.memset(ones1[:], 1.0)

            # residents
            dstw_t = pers.tile([128, ntile], BF16)
            nc.sync.dma_start(dstw_t[:], dstw[:, :])
            batchw_t = pers.tile([128, NWIN], F32)
            nc.sync.dma_start(batchw_t[:], batchw[:, :])
            dinv_all = pers.tile([128, NWIN], F32)
            nc.sync.dma_start(dinv_all[:], dinvw[:, :])
            adw_all = pers.tile([128, NWIN, 10], BF16)
            asad_all = pers.tile([128, NWIN, 10], F32)
            xt_sb = pers.tile([MY_G, 128], F32)
            nc.gpsimd.memset(xt_sb[:], 0.0)
            gcnw_sb = pers.tile([128, 7, HID], BF16)
            nc.sync.dma_start(
                gcnw_sb[:], gcnw.ap().rearrange("(c p) f -> p c f", p=128)
            )

            # ---- phase 1: own h rows (f,h)-major; AllGather table in halves ----
            with (
                tc.tile_pool(name="p1", bufs=1) as p1,
                tc.tile_pool(name="p1h", bufs=3) as p1h,
                tc.tile_pool(name="ps1", bufs=1, space="PSUM") as ps1,
                # protein pools (shared scope so it can fill AllGather #1 time)
                tc.tile_pool(name="pp", bufs=1) as pp,
                tc.tile_pool(name="ppg", bufs=2) as ppg,
                tc.tile_pool(name="psE", bufs=1, space="PSUM") as psE,
                tc.tile_pool(name="psC", bufs=2, space="PSUM") as psC,
                tc.tile_pool(name="psTr", bufs=1, space="PSUM") as psTr,
            ):
                xT_sb = p1.tile([78, NPAD], BF16)
                nc.sync.dma_start(xT_sb[:], xTc[:, :])
                wp_sb = p1.tile([78, 1024], BF16)
                nc.sync.dma_start(wp_sb[:], wpack[:, :])

                for w in range(NWIN):
                    hp = ps1.tile([128, 1024], F32, space="PSUM", tag="hp")
                    for n0, nn in ((0, 512), (512, 288)):
                        nc.tensor.matmul(
                            hp[:, n0 : n0 + nn],
                            lhsT=xT_sb[:, w * 128 : (w + 1) * 128],
                            rhs=wp_sb[:, n0 : n0 + nn],
                            start=True,
                            stop=True,
                        )
                    hst = p1h.tile([128, 800], F32, tag="hst")
                    nc.scalar.copy(hst[:], hp[:, 0:800])
                    hrow = p1h.tile([128, RBF], BF16, tag="hrow")
                    nc.vector.tensor_copy(hrow[:, 0:HID], hst[:, 0:HID])
                    nc.vector.tensor_copy(
                        hrow[:, 780:800].bitcast(F32), hst[:, 780:790]
                    )
                    nc.gpsimd.memset(hrow[:, 800:RBF], 0.0)
                    nc.vector.tensor_tensor(
                        asad_all[:, w, :],
                        hst[:, 780:790],
                        hst[:, 790:800],
                        op=OP.add,
                    )
                    nc.vector.tensor_copy(adw_all[:, w, :], hst[:, 790:800])
                    if w < HALFW:
                        nc.sync.dma_start(
                            hinA.ap()[w * 128 : (w + 1) * 128, :], hrow[:]
                        )
                    else:
                        nc.sync.dma_start(
                            hinB.ap()[(w - HALFW) * 128 : (w - HALFW + 1) * 128, :],
                            hrow[:],
                        )
                    if w == HALFW - 1:
                        nc.gpsimd.collective_compute(
                            "AllGather", OP.bypass, replica_groups=RG,
                            ins=[hinA.ap().opt()],
                            outs=[htabG.ap()[0:SEG, :].opt()],
                        )
                    if w == NWIN - 1:
                        nc.gpsimd.collective_compute(
                            "AllGather", OP.bypass, replica_groups=RG,
                            ins=[hinB.ap().opt()],
                            outs=[htabG.ap()[SEG : 2 * SEG, :].opt()],
                        )

                def bcast_bias(dram, width, name, dt=F32):
                    t = pers.tile([128, width], dt, tag=f"bc_{name}")
                    row = pers.tile([1, width], F32, tag=f"br_{name}")
                    nc.sync.dma_start(row[:], dram[0:1, :])
                    for n0 in range(0, width, 512):
                        nn = min(512, width - n0)
                        ps = psTr.tile([128, 512], F32, space="PSUM", tag="bcps")
                        nc.tensor.matmul(
                            ps[:, :nn], lhsT=ones1[:], rhs=row[:, n0 : n0 + nn],
                            start=True, stop=True,
                        )
                        nc.any.tensor_copy(t[:, n0 : n0 + nn], ps[:, :nn])
                    return t

                gatb_bc = bcast_bias(gatb, HID, "gatb")
                gcnb_bc = bcast_bias(gcn_b, HID, "gcnb")
                fcg1b_bc = bcast_bias(fcg1_b, 1500, "fcg1b", BF16)
                fcg2b_bc = bcast_bias(fcg2_b, 128, "fcg2b")
                fxtb_bc = bcast_bias(fxt_b, 128, "fxtb")
                f1b_bc = bcast_bias(f1_b, 1024, "f1b", BF16)
                f2b_bc = bcast_bias(f2_b, 512, "f2b", BF16)
                f3b_bc = bcast_bias(f3_b, 256, "f3b", BF16)
                f4b_bc = bcast_bias(f4_b, 128, "f4b")
                ob_bc = bcast_bias(o_b, 1, "ob")


                # ---- protein branch (fills the AllGather #1 window) ----
                emb_sb = pp.tile([VOC, EMB], BF16)
                nc.sync.dma_start(emb_sb[:], emb_bf.ap()[:, :])
                cwt_sb = pp.tile([128, KS, NF], BF16)
                nc.sync.dma_start(cwt_sb[:], cwt_bf.ap().rearrange("k p o -> p k o"))
                cb_sb = pp.tile([NF, 1], F32)
                nc.sync.dma_start(cb_sb[:], cb.ap()[:, :])
                cT_all = pers.tile([128, 8, NF, BPC], BF16)

                for grp in range(5):
                    oh = ppg.tile([VOC, 5120], BF16, tag="oh")
                    nc.sync.dma_start(
                        oh[:], onehot.ap()[:, grp * 5120 : (grp + 1) * 5120]
                    )
                    et5 = ppg.tile([128, 5120], BF16, tag="et5")
                    for i in range(10):
                        eps_ = psE.tile([128, 512], F32, space="PSUM", tag="embps")
                        nc.tensor.matmul(
                            eps_[:], lhsT=emb_sb[:],
                            rhs=oh[:, i * 512 : (i + 1) * 512],
                            start=True, stop=True,
                        )
                        nc.any.tensor_copy(et5[:, i * 512 : (i + 1) * 512], eps_[:])
                    for bl in range(5):
                        b = grp * 5 + bl
                        boff = bl * 1000
                        csb = pp.tile([NF, 1024], F32, tag="csb")
                        for p0 in (0, 512):
                            cps = psC.tile([NF, 512], F32, space="PSUM", tag="cps")
                            for k in range(KS):
                                nc.tensor.matmul(
                                    cps[:, 0:512],
                                    lhsT=cwt_sb[:, k, :],
                                    rhs=et5[:, boff + k + p0 : boff + k + p0 + 512],
                                    start=(k == 0),
                                    stop=(k == KS - 1),
                                )
                            nc.scalar.activation(
                                csb[:, p0 : p0 + 512], cps[:, 0:512],
                                AF.Identity, bias=cb_sb[:, 0:1],
                            )
                        for pc in range(8):
                            trc = psTr.tile([128, 128], F32, space="PSUM", tag="trc")
                            nc.tensor.transpose(
                                trc[:, 0:NF],
                                csb[:, pc * 128 : (pc + 1) * 128],
                                ident[0:NF, 0:NF],
                            )
                            nc.any.tensor_copy(cT_all[:, pc, :, b], trc[:, 0:NF])

            # shared edge-phase machinery -----------------------------------
            def edge_phase(table, gat, epilogue, extra_psum):
                with (
                    tc.tile_pool(name="msgp", bufs=3) as msgp,
                    tc.tile_pool(name="smallp", bufs=2) as smallp,
                    tc.tile_pool(name="maskp", bufs=3) as maskp,
                    tc.tile_pool(name="epip", bufs=2) as epip,
                    tc.tile_pool(name="hop", bufs=2) as hop,
                    tc.tile_pool(name="idxp", bufs=1) as idxp,
                    tc.tile_pool(name="psA", bufs=2, space="PSUM") as psA,
                    tc.tile_pool(name="psS", bufs=1, space="PSUM") as psS,
                    tc.tile_pool(name="psD", bufs=1, space="PSUM") as psD,
                    extra_psum(tc) as psH,
                ):
                    aggp = None
                    hown = {}
                    idx_t = idxp.tile([128, epc // 16], I16, tag="idx")
                    nc.sync.dma_start(
                        idx_t[:], (src16 if gat else src16b).ap()[:, :]
                    )
                    for c in range(nchunk):
                        T = min(16, ntile - c * 16)
                        msg = msgp.tile([128, 16, RBF], BF16, tag="msg")
                        nc.gpsimd.dma_gather(
                            msg[:, 0:T, :],
                            table.ap()[:, 0:RBF],
                            idx_t[:, c * 128 : c * 128 + T * 8],
                            num_idxs=T * 128,
                            num_idxs_reg=T * 128,
                            elem_size=RBF,
                            elem_step=RBF,
                            single_packet=False,
                        )
                        maskall = maskp.tile([128, 16, 128], BF16, tag="maskall")
                        nc.vector.tensor_tensor(
                            maskall[:, 0:T, :],
                            dstw_t[:, c * 16 : c * 16 + T, None].to_broadcast(
                                [128, T, 128]
                            ),
                            iota_bf[:].to_broadcast([128, T, 128]),
                            op=OP.is_equal,
                        )
                        if gat:
                            sall = smallp.tile([128, 16, 10], F32, tag="sall")
                            sl2 = smallp.tile([128, 16, 10], F32, tag="sl2")
                            mTc = maskp.tile([128, 16, 128], BF16, tag="mTc")
                            nc.sync.dma_start(
                                mTc[:, 0:T, :],
                                maskTd.ap()[:, c * 2048 : c * 2048 + T * 128]
                                .rearrange("p (t e) -> p t e", e=128),
                            )
                            adx = psD.tile([128, 512], F32, space="PSUM", tag="adx")
                            for j in range(T):
                                g = c * 16 + j
                                nc.tensor.matmul(
                                    adx[:, j * 10 : j * 10 + 10],
                                    lhsT=mTc[:, j, :],
                                    rhs=adw_all[:, g // tpw, :],
                                    start=True,
                                    stop=True,
                                )
                            nc.vector.tensor_tensor(
                                sall[:, 0:T, :],
                                msg[:, 0:T, 780:800].bitcast(F32),
                                adx[:, 0 : T * 10].rearrange("p (a b) -> p a b", b=10),
                                op=OP.add,
                            )
                            nc.vector.tensor_scalar_mul(
                                sl2[:, 0:T, :], sall[:, 0:T, :], 0.2
                            )
                            nc.vector.tensor_tensor(
                                sl2[:, 0:T, :], sall[:, 0:T, :], sl2[:, 0:T, :],
                                op=OP.max,
                            )
                            nc.scalar.activation(
                                msg[:, 0:T, 780:790], sl2[:, 0:T, :], AF.Exp
                            )
                            nc.vector.tensor_tensor(
                                msg[:, 0:T, 0:HID].rearrange(
                                    "p c (f h) -> p c f h", h=H
                                ),
                                msg[:, 0:T, 0:HID].rearrange(
                                    "p c (f h) -> p c f h", h=H
                                ),
                                msg[:, 0:T, 780:790][:, :, None, :].to_broadcast(
                                    [128, T, F, H]
                                ),
                                op=OP.mult,
                            )
                        # pass 2: scatter matmuls
                        n_hi = 790 if gat else HID
                        for j in range(T):
                            g = c * 16 + j
                            w, r = divmod(g, tpw)
                            if r == 0:
                                aggp = psA.tile(
                                    [128, 1024], F32, space="PSUM", tag="aggp"
                                )
                                ht = hop.tile([128, RBF], BF16, tag="hown")
                                if gat:
                                    rb = hinA if w < HALFW else hinB
                                    ro = (w % HALFW) * 128
                                else:
                                    rb = agins[w // 5]
                                    ro = (w % 5) * 128
                                nc.sync.dma_start(ht[:], rb.ap()[ro : ro + 128, :])
                                hown[w] = ht
                            for n0, nn in ((0, 512), (512, n_hi - 512)):
                                nc.tensor.matmul(
                                    aggp[:, n0 : n0 + nn],
                                    lhsT=maskall[:, j, :],
                                    rhs=msg[:, j, n0 : n0 + nn],
                                    start=(r == 0),
                                    stop=(r == tpw - 1),
                                )
                            if r == tpw - 1:
                                epilogue(w, aggp, epip, hown.pop(w), psH, psS)

            # ---- phase 2: GAT edge phase (h2 + AllGather #2 interleaved) ----
            def gat_epilogue(w, aggp, epip, hown, psH, psS):
                exs1 = epip.tile([128, 10], F32, tag="exs1")
                nc.vector.tensor_scalar_mul(exs1[:], asad_all[:, w, :], 0.2)
                nc.vector.tensor_tensor(
                    exs1[:], asad_all[:, w, :], exs1[:], op=OP.max
                )
                exs2 = epip.tile([128, 10], F32, tag="exs2")
                nc.scalar.activation(exs2[:], exs1[:], AF.Exp)
                rec = epip.tile([128, 10], F32, tag="rec")
                nc.vector.tensor_tensor(rec[:], aggp[:, 780:790], exs2[:], op=OP.add)
                rcp = epip.tile([128, 10], F32, tag="rcp")
                nc.vector.reciprocal(rcp[:], rec[:])
                selfm = epip.tile([128, HID], F32, tag="selfm")
                nc.vector.tensor_tensor(
                    selfm[:].rearrange("p (f h) -> p f h", h=H),
                    hown[:, 0:HID].rearrange("p (f h) -> p f h", h=H),
                    exs2[:, None, :].to_broadcast([128, F, H]),
                    op=OP.mult,
                )
                x1s = selfm
                nc.vector.tensor_tensor(x1s[:], aggp[:, 0:HID], selfm[:], op=OP.add)
                nc.vector.tensor_tensor(
                    x1s[:].rearrange("p (f h) -> p f h", h=H),
                    x1s[:].rearrange("p (f h) -> p f h", h=H),
                    rcp[:, None, :].to_broadcast([128, F, H]),
                    op=OP.mult,
                )
                nc.vector.tensor_tensor(x1s[:], x1s[:], gatb_bc[:, 0:HID], op=OP.add)
                # x1w = relu(x1) * dinv in one ACT op (dinv>0 commutes with relu);
                # the pre-scaled rows make h2 = x1w @ gcn_W come out dinv-scaled.
                x1w = epip.tile([128, HID], BF16, tag="x1w")
                nc.scalar.activation(
                    x1w[:], x1s[:], AF.Relu, scale=dinv_all[:, w : w + 1]
                )
                # h2 = (relu(x1)*dinv) @ gcn_W, written to agin
                x1T = epip.tile([128, 7, 128], BF16, tag="x1T")
                for kc in range(7):
                    sz = 128 if kc < 6 else 12
                    trp = psS.tile([128, 512], BF16, space="PSUM", tag="trT")
                    nc.tensor.transpose(
                        trp[0:sz, 0:128], x1w[:, kc * 128 : kc * 128 + sz], ident_bf[:]
                    )
                    nc.any.tensor_copy(x1T[0:sz, kc, :], trp[0:sz, 0:128])
                h2ps = psH.tile([128, 1024], F32, space="PSUM", tag="h2ps")
                for kc in range(7):
                    sz = 128 if kc < 6 else 12
                    for n0, nn in ((0, 512), (512, 268)):
                        nc.tensor.matmul(
                            h2ps[:, n0 : n0 + nn],
                            lhsT=x1T[0:sz, kc, :],
                            rhs=gcnw_sb[0:sz, kc, n0 : n0 + nn],
                            start=(kc == 0),
                            stop=(kc == 6),
                        )
                h2s = epip.tile([128, RBF], BF16, tag="h2s")
                nc.vector.tensor_copy(h2s[:, 0:HID], h2ps[:, 0:HID])
                seg, soff = w // 5, (w % 5) * 128
                nc.sync.dma_start(
                    agins[seg].ap()[soff : soff + 128, :], h2s[:]
                )
                if w % 5 == 4:
                    SEGQ = NCORES * 640
                    nc.gpsimd.collective_compute(
                        "AllGather", OP.bypass, replica_groups=RG,
                        ins=[agins[seg].ap().opt()],
                        outs=[htab2G.ap()[seg * SEGQ : (seg + 1) * SEGQ, :].opt()],
                    )

            def psH_gat(tc):
                return tc.tile_pool(name="psH", bufs=1, space="PSUM")

            edge_phase(htabG, True, gat_epilogue, psH_gat)

            # ---- fxt (protein tail) fills the AllGather #2 drain window ----
            with (
                tc.tile_pool(name="ppw2", bufs=2) as ppw2,
                tc.tile_pool(name="psX2", bufs=1, space="PSUM") as psX2,
            ):
                xtps = psX2.tile([MY_G, 128], F32, space="PSUM", tag="xtps")
                for sc in range(16):
                    wpt = ppw2.tile([128, 16, 128], BF16, tag="wpt")
                    nc.sync.dma_start(
                        wpt[:],
                        fxt_Wp.ap()[sc * 2048 : (sc + 1) * 2048, :].rearrange(
                            "(c p) j -> p c j", p=128
                        ),
                    )
                    for sub in range(16):
                        q = sc * 16 + sub
                        o, t8 = q // 8, q % 8
                        nc.tensor.matmul(
                            xtps[0:BPC, :],
                            lhsT=cT_all[:, t8, o, :],
                            rhs=wpt[:, sub, :],
                            start=(q == 0),
                            stop=(q == 255),
                        )
                nc.vector.tensor_tensor(
                    xt_sb[0:BPC, :], xtps[0:BPC, :], fxtb_bc[0:BPC, :], op=OP.add
                )

            # ---- phase 3: GCN edge phase + pooling; phase 4: head ----
            with tc.tile_pool(name="psP", bufs=1, space="PSUM") as psP:
                poolps = psP.tile([GSLOT, 1024], F32, space="PSUM", tag="poolps")

                def gcn_epilogue(w, aggp, epip, h2own, psH, psS):
                    x2s = epip.tile([128, HID], F32, tag="x2s")
                    nc.vector.tensor_tensor(
                        x2s[:], aggp[:, 0:HID], h2own[:, 0:HID], op=OP.add
                    )
                    x2d = epip.tile([128, HID], F32, tag="x2d")
                    nc.scalar.activation(
                        x2d[:], x2s[:], AF.Identity, scale=dinv_all[:, w : w + 1]
                    )
                    nc.vector.tensor_tensor(
                        x2d[:], x2d[:], gcnb_bc[:, 0:HID], op=OP.add
                    )
                    x2w = x2s
                    nc.vector.tensor_scalar_max(x2w[:], x2d[:], 0.0)
                    ph = epip.tile([128, GSLOT], F32, tag="poolhot")
                    nc.vector.tensor_tensor(
                        ph[:],
                        batchw_t[:, w : w + 1].to_broadcast([128, GSLOT]),
                        iota_f[:, 0:GSLOT],
                        op=OP.is_equal,
                    )
                    for n0, nn in ((0, 512), (512, 268)):
                        nc.tensor.matmul(
                            poolps[:, n0 : n0 + nn],
                            lhsT=ph[:],
                            rhs=x2w[:, n0 : n0 + nn],
                            start=(w == 0),
                            stop=(w == NWIN - 1),
                        )

                def psH_gcn(tc):
                    return tc.tile_pool(name="psH2", bufs=1, space="PSUM")

                edge_phase(htab2G, False, gcn_epilogue, psH_gcn)
                poolsb = pers.tile([GSLOT, HID], F32)
                nc.any.tensor_copy(poolsb[:], poolps[:, 0:HID])

            with (
                tc.tile_pool(name="p5", bufs=1) as p5,
                tc.tile_pool(name="p5w", bufs=2) as p5w,
                tc.tile_pool(name="ps5", bufs=2, space="PSUM") as ps5,
                tc.tile_pool(name="ps5t", bufs=2, space="PSUM") as ps5t,
            ):
                nc.sync.dma_start(poolin.ap()[:, :], poolsb[:])
                nc.gpsimd.collective_compute(
                    "AllGather", OP.bypass, replica_groups=RG,
                    ins=[poolin.ap().opt()],
                    outs=[poolall.ap().opt()],
                )
                Cc_sb = p5.tile([128, 4, MY_G], F32)
                nc.sync.dma_start(
                    Cc_sb[:], Cc.ap().rearrange("(c p) g -> p c g", p=128)
                )
                pall = p5.tile([128, 4, HID], F32)
                nc.sync.dma_start(
                    pall[:], poolall.ap().rearrange("(c p) f -> p c f", p=128)
                )
                xgps = ps5.tile([MY_G, 1024], F32, space="PSUM", tag="mlp_ps")
                for kc in range(4):
                    for n0, nn in ((0, 512), (512, 268)):
                        nc.tensor.matmul(
                            xgps[:, n0 : n0 + nn],
                            lhsT=Cc_sb[:, kc, :],
                            rhs=pall[:, kc, n0 : n0 + nn],
                            start=(kc == 0),
                            stop=(kc == 3),
                        )
                xg = p5.tile([MY_G, HID], F32, tag="act0")
                nc.any.tensor_copy(xg[:], xgps[:, 0:HID])

                def dense(x_sb, k_real, w_dram, w_rows, n_out, b_bc, relu, tag):
                    nkc = (k_real + 127) // 128
                    xT_t = p5.tile([128, nkc, MY_G], F32, tag="xT5")
                    for kc in range(nkc):
                        sz = min(128, k_real - kc * 128)
                        trp = ps5t.tile([128, MY_G], F32, space="PSUM", tag="tr5")
                        nc.tensor.transpose(
                            trp[0:sz, :],
                            x_sb[:, kc * 128 : kc * 128 + sz],
                            ident[0:MY_G, 0:MY_G],
                        )
                        nc.any.tensor_copy(xT_t[0:sz, kc, :], trp[0:sz, :])
                    w_sb = p5w.tile([128, w_rows // 128, n_out], F32, tag="w5")
                    nc.sync.dma_start(
                        w_sb[:], w_dram.ap().rearrange("(c p) f -> p c f", p=128)
                    )
                    yps = ps5.tile([MY_G, 1536], F32, space="PSUM", tag="mlp_ps")
                    for n0 in range(0, n_out, 512):
                        nn = min(512, n_out - n0)
                        for kc in range(nkc):
                            sz = min(128, k_real - kc * 128)
                            nc.tensor.matmul(
                                yps[:, n0 : n0 + nn],
                                lhsT=xT_t[0:sz, kc, :],
                                rhs=w_sb[0:sz, kc, n0 : n0 + nn],
                                start=(kc == 0),
                                stop=(kc == nkc - 1),
                            )
                    yf = p5.tile([MY_G, n_out], F32, tag="y5f")
                    nc.vector.tensor_tensor(
                        yf[:], yps[:, 0:n_out], b_bc[0:MY_G, 0:n_out], op=OP.add
                    )
                    y = p5.tile([MY_G, n_out], F32, tag="y5")
                    nc.scalar.activation(
                        y[:], yf[:], AF.Relu if relu else AF.Identity
                    )
                    return y, yf

                y1, _ = dense(xg, HID, fcg1_Wp, 896, 1500, fcg1b_bc, True, "fcg1")
                xgo, _ = dense(y1, 1500, fcg2_Wp, 1536, 128, fcg2b_bc, False, "fcg2")
                xc = p5.tile([MY_G, 256], F32, tag="xc")
                nc.any.tensor_copy(xc[:, 0:128], xgo[:])
                nc.any.tensor_copy(xc[:, 128:256], xt_sb[:])
                a1, _ = dense(xc, 256, f1_W, 256, 1024, f1b_bc, True, "f1")
                a2, _ = dense(a1, 1024, f2_W, 1024, 512, f2b_bc, True, "f2")
                a3, _ = dense(a2, 512, f3_W, 512, 256, f3b_bc, True, "f3")
                a4, _ = dense(a3, 256, f4_W, 256, 128, f4b_bc, True, "f4")
                _, yof = dense(a4, 128, o_W, 128, 1, ob_bc, False, "o")
                nc.sync.dma_start(out_d.ap()[:, :], yof[:])
                if KDEBUG:
                    dbg = p5.tile([128, 2, RBF], BF16, tag="dbg")
                    nc.sync.dma_start(
                        dbg[:], htabG.ap()[0:256, :].rearrange("(c p) f -> p c f", p=128)
                    )
                    nc.sync.dma_start(
                        out_h.ap().rearrange("(c p) f -> p c f", p=128), dbg[:]
                    )
                    dbg2 = p5.tile([128, 2, RBF], BF16, tag="dbg2")
                    nc.sync.dma_start(
                        dbg2[:], htab2G.ap()[0:256, :].rearrange("(c p) f -> p c f", p=128)
                    )
                    nc.sync.dma_start(
                        out_h2.ap().rearrange("(c p) f -> p c f", p=128), dbg2[:]
                    )
                    dbg3 = p5.tile([128, 4, HID], F32, tag="dbg3")
                    nc.sync.dma_start(
                        dbg3[:], poolall.ap().rearrange("(c p) f -> p c f", p=128)
                    )
                    nc.sync.dma_start(
                        out_pool.ap().rearrange("(c p) f -> p c f", p=128), dbg3[:]
                    )
                    nc.sync.dma_start(out_dinv.ap()[:, :], dinv_all[:].rearrange("p a -> p a"))
                    nc.sync.dma_start(out_xt.ap()[:, :], xt_sb[:])

    nc.compile()
    _CACHE[key] = nc
    return nc


# ---------------------------------------------------------------- entry point


def _ensure_ntff_hook():
    """Install antenv.axon_hooks + register the ctypes NTFF hook if the image
    lacks them (profiling only; failures are non-fatal)."""
    import types

    try:
        import antenv.axon_hooks  # noqa: F401

        if antenv.axon_hooks.get_axon_ntff_profile_hook() is not None:
            return
    except ImportError:
        import antenv

        mod = types.ModuleType("antenv.axon_hooks")
        mod._hook = None

        def set_axon_ntff_profile_hook(h, _m=mod):
            _m._hook = h

        def get_axon_ntff_profile_hook(_m=mod):
            return _m._hook

        mod.set_axon_ntff_profile_hook = set_axon_ntff_profile_hook
        mod.get_axon_ntff_profile_hook = get_axon_ntff_profile_hook
        sys.modules["antenv.axon_hooks"] = mod
        antenv.axon_hooks = mod
    try:
        from antenv.axon_hooks import set_axon_ntff_profile_hook as _set
        from trn_agent_boot.trn_boot import _ntff_profile_via_ctypes

        hook = _ntff_profile_via_ctypes("/opt/axon/libaxon_pjrt.so")
        if hook is not None:
            _set(hook)
    except Exception:
        pass


def kernel(**inputs) -> np.ndarray:
    per_core, meta = host_prep(inputs)
    nc = build_bass(meta)
    in_maps = [{k: np.ascontiguousarray(v) for k, v in d.items()} for d in per_core]
    trace = bool(int(os.environ.get("KERNEL_TRACE", "0")))
    if trace:
        _ensure_ntff_hook()
    res = run_bass_kernel_spmd(nc, in_maps, core_ids=list(range(NCORES)), trace=trace)
    if trace and res.exec_time_ns is not None:
        print(f"HW exec time: {res.exec_time_ns} ns")
        kernel.last_exec_ns = res.exec_time_ns
    out = np.concatenate([res.results[c]["out"][:BPC] for c in range(NCORES)], 0)
    return out.astype(np.float32)
